# revision 1
# baseline (speedup 1.0000x reference)
"""Trainium2 Bass kernel for nn_DAGLinkPredictor (3-layer GAT + edge decoder).

Sharding: dst-node-sharded GAT across 8 cores. Edges (incl self-loops) are
sorted by dst and grouped into per-core 128-node dst blocks. Per block:
  - dma_gather pulls [h | al_src] rows (bf16) for edge sources from the
    replicated node table T_l (two gathers: src < 32768 and >= 32768, since
    gather indices are int16),
  - a one-hot scatter matrix S (built on VectorE from dst offsets vs an iota
    row) scatter-adds messages into PSUM via TensorE,
  - softmax is denominator-style: out[d] = sum_e exp(lrelu(als+ald)) * h[src]
    / sum_e exp(...), which is exactly segment-softmax without segment-max
    (mathematically identical, and safe here since logits are O(1)).
Node phase (per block): divide, elu, transpose, dense matmul with the next
layer's folded weights -> next table slice. Slices are AllGathered between
layers. Decoder: transpose-gathers of z rows + dense matmuls, trans_bias via
gathered rows x one-hot reduce.
"""
import numpy as np
import ml_dtypes

N = 50000
NP = 50176            # padded nodes: 8 * 6272
SLICE = NP // 8       # 6272 nodes per core
NB = SLICE // 128     # 49 blocks per core
E = 800000
EL = 100000
HALF = 32768          # int16 gather index limit
NTYPES = 311
EMB = 16
COMB = 48

# layer configs: (Din, HD, H, D, src_elem(row stride), dst_off_elems)
LCFG = [
    (48, 256, 4, 64, 384, 256),
    (256, 256, 2, 128, 384, 256),
    (256, 128, 1, 128, 256, 128),
]
TDEC_W = 128          # decode table row elems (bf16, 256B)
TB_W = 384            # padded trans_bias row (bf16)
DEC_TILE = 512

bf16 = ml_dtypes.bfloat16


def _wrap_idx(vals):
    """int16 index array for dma_gather: [128, n/16], wrapped over 16
    partitions and replicated across the 8 gpsimd cores."""
    n = len(vals)
    assert n % 16 == 0
    a = np.zeros((128, n // 16), np.int16)
    v = np.asarray(vals, np.int64)
    assert v.min() >= 0 and v.max() < 32768
    w = v.reshape(n // 16, 16).T.astype(np.int16)  # [16, n/16]
    for g in range(8):
        a[16 * g:16 * g + 16, :] = w
    return a


def _slotmajor(vals, fill, dtype):
    """[128, n/128] array with element (p, c) = vals[c*128+p]."""
    n = len(vals)
    assert n % 128 == 0
    return np.asarray(vals, np.float64).reshape(n // 128, 128).T.astype(dtype)


def prep(x, edge_index, edge_label_index, emb, W1, a_src1, a_dst1, b1,
         W2, a_src2, a_dst2, b2, W3, a_src3, a_dst3, b3,
         Wl1, bl1, Wl2, bl2, trans_bias):
    """Host-side (integer/index + weight-layout) preprocessing."""
    types = x[:, 0].astype(np.int64)

    # --- weight folds: RHS_l = [W_l | W_l@a_src per head | W_l@a_dst] ---
    def fold(W, a_s, a_d, H, D):
        cols_s = np.stack([W[:, h * D:(h + 1) * D] @ a_s[h] for h in range(H)], 1)
        cols_d = np.stack([W[:, h * D:(h + 1) * D] @ a_d[h] for h in range(H)], 1)
        return np.concatenate([W, cols_s, cols_d], 1).astype(np.float32)
    RHS = [fold(W1, a_src1, a_dst1, 4, 64),
           fold(W2, a_src2, a_dst2, 2, 128),
           fold(W3, a_src3, a_dst3, 1, 128)]

    emb_pad = np.zeros((NTYPES, 64), np.float32)
    emb_pad[:, :EMB] = emb
    TBpad = np.zeros((NTYPES, TB_W), bf16)
    TBpad[:, :NTYPES] = trans_bias.astype(bf16)

    # --- edges: add self loops, sort by dst, bucket per core / block ---
    loops = np.arange(N, dtype=np.int64)
    src = np.concatenate([edge_index[0].astype(np.int64), loops])
    dst = np.concatenate([edge_index[1].astype(np.int64), loops])
    order = np.argsort(dst, kind="stable")
    src, dst = src[order], dst[order]

    blk = dst // 128          # global block id (0..391)
    # per (core, block-in-core, half) edge lists
    per = [[[None, None] for _ in range(NB)] for _ in range(8)]
    for c in range(8):
        for b in range(NB):
            gb = c * NB + b
            m = blk == gb
            s, d = src[m], dst[m]
            lo = s < HALF
            per[c][b][0] = (s[lo], d[lo])
            per[c][b][1] = (s[~lo] - HALF, d[~lo])
    # chunk counts, shared across cores (same kernel structure)
    CA = [max(1, max((len(per[c][b][0][0]) + 127) // 128 for c in range(8)))
          for b in range(NB)]
    CB = [max((len(per[c][b][1][0]) + 127) // 128 for c in range(8))
          for b in range(NB)]

    idxA, idxB, dsti, doff = [], [], [], []
    for c in range(8):
        la, lb, ld, lo = [], [], [], []
        for b in range(NB):
            for half, (cnt, acc) in (((0), (CA[b], la)), ((1), (CB[b], lb))):
                s, d = per[c][b][half]
                ns = cnt * 128
                sp = np.zeros(ns, np.int64)
                sp[:len(s)] = s
                acc.append(sp)
                dl = np.zeros(ns, np.int64)          # dst local to core slice
                dl[:len(d)] = d[:len(d)] - c * SLICE
                ld.append(dl)
                off = np.full(ns, 255, np.int64)     # 255 => padded slot
                off[:len(d)] = d[:len(d)] - (c * SLICE + b * 128)
                lo.append(off)
        idxA.append(_wrap_idx(np.concatenate(la)))
        idxB.append(_wrap_idx(np.concatenate(lb)))
        dsti.append(_wrap_idx(np.concatenate(ld)))
        doff.append(_slotmajor(np.concatenate(lo), 255, np.float32))

    # --- label edges: 4 groups by (ls-half, ld-half), padded per group ---
    ls = edge_label_index[0].astype(np.int64)
    ld_ = edge_label_index[1].astype(np.int64)
    elpc = (EL + 7) // 8                      # 12500 label edges per core
    groups_sz = np.zeros((8, 4), np.int64)
    per_dec = [[None] * 4 for _ in range(8)]
    for c in range(8):
        lo_, hi_ = c * elpc, min((c + 1) * elpc, EL)
        eidx = np.arange(lo_, hi_)
        g = (ls[eidx] >= HALF).astype(np.int64) * 2 + (ld_[eidx] >= HALF)
        for gi in range(4):
            per_dec[c][gi] = eidx[g == gi]
            groups_sz[c, gi] = len(per_dec[c][gi])
    GSZ = [int(-(-groups_sz[:, gi].max() // DEC_TILE) * DEC_TILE)
           for gi in range(4)]
    SL = sum(GSZ)
    lsw, ldw, tlsw, tldw, slotmap = [], [], [], [], []
    for c in range(8):
        a_ls = np.zeros(SL, np.int64)
        a_ld = np.zeros(SL, np.int64)
        a_tls = np.zeros(SL, np.int64)
        a_tld = np.zeros(SL, np.int64)
        smap = np.full(SL, -1, np.int64)
        pos = 0
        for gi in range(4):
            e = per_dec[c][gi]
            n = len(e)
            a_ls[pos:pos + n] = ls[e] - (HALF if gi >= 2 else 0)
            a_ld[pos:pos + n] = ld_[e] - (HALF if gi % 2 else 0)
            a_tls[pos:pos + n] = types[np.minimum(ls[e], N - 1)]
            a_tld[pos:pos + n] = types[np.minimum(ld_[e], N - 1)]
            smap[pos:pos + n] = e
            pos += GSZ[gi]
        lsw.append(_wrap_idx(a_ls))
        ldw.append(_wrap_idx(a_ld))
        tlsw.append(_wrap_idx(a_tls))
        tldw.append(_slotmajor(a_tld, 0, np.float32))
        slotmap.append(smap)

    iota128 = np.tile(np.arange(128, dtype=np.float32)[None, :], (128, 1))
    iota384 = np.tile(np.arange(TB_W, dtype=np.float32)[None, :], (128, 1))
    ident = np.eye(128, dtype=np.float32)

    in_maps = []
    for c in range(8):
        xr = np.zeros((SLICE, 33), np.float32)
        n0 = c * SLICE
        n1 = min((c + 1) * SLICE, N)
        if n1 > n0:
            xr[:n1 - n0] = x[n0:n1]
        ti = np.zeros(SLICE, np.int64)
        if n1 > n0:
            ti[:n1 - n0] = types[n0:n1]
        in_maps.append(dict(
            x_rows=xr, embt_idx=_wrap_idx(ti),
            emb_pad=emb_pad,
            RHS1=RHS[0], RHS2=RHS[1], RHS3=RHS[2],
            idxA=idxA[c], idxB=idxB[c], dsti=dsti[c], doff=doff[c],
            ls_idx=lsw[c], ld_idx=ldw[c], tls_idx=tlsw[c], tld=tldw[c],
            iota128=iota128, iota384=iota384, ident=ident,
            TBpad=TBpad,
            Wl1a=Wl1[:128].astype(bf16), Wl1b=Wl1[128:].astype(bf16),
            Wl2=Wl2.astype(bf16), bl1=bl1.reshape(64, 1).astype(np.float32),
        ))
    cfg = dict(CA=CA, CB=CB, GSZ=GSZ, SL=SL,
               SA=sum(CA) * 128, SB=sum(CB) * 128,
               ST=(sum(CA) + sum(CB)) * 128,
               bl2=float(np.asarray(bl2).reshape(-1)[0]),
               b=[np.asarray(b1), np.asarray(b2), np.asarray(b3)],
               slotmap=slotmap)
    return in_maps, cfg


# ---------------------------------------------------------------- golden ---
def golden(in_maps, cfg):
    """numpy mirror of the device algorithm (fp32; layout-accurate)."""
    CA, CB = cfg["CA"], cfg["CB"]
    T = None
    out_all = []
    for li, (Din, HD, H, D, STRIDE, OFF) in enumerate(LCFG):
        Tn = np.zeros((NP, STRIDE), np.float32)
        slices = []
        for c in range(8):
            im = in_maps[c]
            if li == 0:
                embg = im["emb_pad"][_unwrap(im["embt_idx"], SLICE)][:, :EMB]
                x0 = np.concatenate([embg, im["x_rows"][:, 1:33]], 1)
                hrow = x0 @ im["RHS1"]
            else:
                hrow = PREV[c] @ im[f"RHS{li + 1}"]
            sl = np.zeros((SLICE, STRIDE), np.float32)
            sl[:, :hrow.shape[1]] = hrow
            slices.append(sl)
            Tn[c * SLICE:(c + 1) * SLICE] = sl
        T = Tn.astype(bf16).astype(np.float32)
        slices = [s.astype(bf16).astype(np.float32) for s in slices]
        # edge phase
        PREV = []
        for c in range(8):
            im = in_maps[c]
            ia = _unwrap(im["idxA"], cfg["SA"])
            ib = _unwrap(im["idxB"], cfg["SB"])
            idt = _unwrap(im["dsti"], cfg["ST"])
            dof = im["doff"].T.reshape(-1)  # slot order
            xl = np.zeros((SLICE, HD), np.float32)
            pa = pb = pt = 0
            for b in range(NB):
                sA, sB = CA[b] * 128, CB[b] * 128
                gidx = np.concatenate([ia[pa:pa + sA],
                                       ib[pb:pb + sB] + HALF])
                pa += sA; pb += sB
                nsl = sA + sB
                G = T[gidx, :]                     # [nsl, STRIDE]
                GD = slices[c][idt[pt:pt + nsl], OFF:OFF + 128]
                off = dof[pt:pt + nsl]
                pt += nsl
                S = (off[:, None] == np.arange(128)[None, :]).astype(np.float32)
                als = G[:, OFF:OFF + H]
                ald = GD[:, H:2 * H]
                lg = als + ald
                lg = np.where(lg > 0, lg, 0.2 * lg)
                e = np.exp(lg).astype(bf16).astype(np.float32)
                msg = (G[:, :HD].reshape(nsl, H, D) * e[:, :, None]
                       ).reshape(nsl, HD).astype(bf16).astype(np.float32)
                num = S.T @ msg                    # [128, HD]
                den = S.T @ e                      # [128, H]
                r = 1.0 / (den + 1e-16)
                xb = (num.reshape(128, H, D) * r[:, :, None]).reshape(128, HD)
                bvec = cfg["b"][li]
                if np.any(bvec != 0):
                    xb = xb + bvec
                if li < 2:
                    xb = np.maximum(xb, 0) + np.exp(np.minimum(xb, 0)) - 1
                xl[b * 128:(b + 1) * 128] = xb
            PREV.append(xl)
    # decode
    TD = np.zeros((NP, TDEC_W), np.float32)
    for c in range(8):
        TD[c * SLICE:(c + 1) * SLICE] = PREV[c].astype(bf16)
    TD = TD.astype(bf16).astype(np.float32)
    scores = []
    for c in range(8):
        im = in_maps[c]
        lsv = _unwrap(im["ls_idx"], cfg["SL"])
        ldv = _unwrap(im["ld_idx"], cfg["SL"])
        tlsv = _unwrap(im["tls_idx"], cfg["SL"])
        tldv = im["tld"].T.reshape(-1)
        base_ls = np.zeros(cfg["SL"], np.int64)
        base_ld = np.zeros(cfg["SL"], np.int64)
        pos = 0
        for gi in range(4):
            base_ls[pos:pos + cfg["GSZ"][gi]] = HALF if gi >= 2 else 0
            base_ld[pos:pos + cfg["GSZ"][gi]] = HALF if gi % 2 else 0
            pos += cfg["GSZ"][gi]
        zl = TD[lsv + base_ls]
        zr = TD[ldv + base_ld]
        W1a = in_maps[c]["Wl1a"].astype(np.float32)
        W1b = in_maps[c]["Wl1b"].astype(np.float32)
        h = np.maximum(zl @ W1a + zr @ W1b + in_maps[c]["bl1"].T, 0).astype(bf16).astype(np.float32)
        base = h @ in_maps[c]["Wl2"].astype(np.float32) + cfg["bl2"]
        TBg = in_maps[c]["TBpad"].astype(np.float32)[tlsv]
        oh = (tldv[:, None] == np.arange(TB_W)[None, :])
        bias = (TBg * oh).sum(1)
        scores.append(base[:, 0] + bias)
    out = np.zeros((EL, 1), np.float32)
    for c in range(8):
        m = cfg["slotmap"][c] >= 0
        out[cfg["slotmap"][c][m], 0] = scores[c][m]
    return out


def _unwrap(w, n):
    return w[:16, :].T.reshape(-1)[:n].astype(np.int64)


# ----------------------------------------------------------------- device ---
def build(cfg):
    import concourse.bacc as bacc
    import concourse.mybir as mybir
    from concourse.tile import TileContext
    dt = mybir.dt
    F = mybir.ActivationFunctionType
    A = mybir.AluOpType
    CA, CB, SL = cfg["CA"], cfg["CB"], cfg["SL"]
    SA, SB, ST = cfg["SA"], cfg["SB"], cfg["ST"]

    nc = bacc.Bacc(num_devices=8, dynamic_dma_scratch_size=32768)
    GMAX = 6  # max 128-chunks per dma_gather (descriptor-ring bound)

    def gat(out_ap, in_ap, idx_tile, col0, nchunk, elem, **kw):
        for s0 in range(0, nchunk, GMAX):
            s1 = min(s0 + GMAX, nchunk)
            nc.gpsimd.dma_gather(
                out_ap[:, s0:s1, :], in_ap,
                idx_tile[:, col0 + s0 * 8: col0 + s1 * 8],
                (s1 - s0) * 128, (s1 - s0) * 128, elem, **kw)
    inp = {}
    for name, shape, d in [
        ("x_rows", [SLICE, 33], dt.float32),
        ("embt_idx", [128, SLICE // 16], dt.int16),
        ("emb_pad", [NTYPES, 64], dt.float32),
        ("RHS1", [48, 264], dt.float32),
        ("RHS2", [256, 260], dt.float32),
        ("RHS3", [256, 130], dt.float32),
        ("idxA", [128, SA // 16], dt.int16),
        ("idxB", [128, SB // 16], dt.int16),
        ("dsti", [128, ST // 16], dt.int16),
        ("doff", [128, ST // 128], dt.float32),
        ("ls_idx", [128, SL // 16], dt.int16),
        ("ld_idx", [128, SL // 16], dt.int16),
        ("tls_idx", [128, SL // 16], dt.int16),
        ("tld", [128, SL // 128], dt.float32),
        ("iota128", [128, 128], dt.float32),
        ("iota384", [128, TB_W], dt.float32),
        ("ident", [128, 128], dt.float32),
        ("TBpad", [NTYPES, TB_W], dt.bfloat16),
        ("Wl1a", [128, 64], dt.bfloat16),
        ("Wl1b", [128, 64], dt.bfloat16),
        ("Wl2", [64, 1], dt.bfloat16),
        ("bl1", [64, 1], dt.float32),
    ]:
        inp[name] = nc.dram_tensor(name, shape, d, kind="ExternalInput")
    score_out = nc.dram_tensor("score", [SL, 1], dt.float32, kind="ExternalOutput")

    sl_t = [nc.dram_tensor(f"slice{l}", [SLICE, LCFG[l][4]], dt.bfloat16,
                           kind="Internal") for l in range(3)]
    sl_d = nc.dram_tensor("sliceD", [SLICE, TDEC_W], dt.bfloat16, kind="Internal")
    T_t = [nc.dram_tensor(f"T{l}", [NP, LCFG[l][4]], dt.bfloat16,
                          kind="Internal", addr_space="Shared") for l in range(3)]
    T_d = nc.dram_tensor("TD", [NP, TDEC_W], dt.bfloat16,
                         kind="Internal", addr_space="Shared")

    with TileContext(nc, num_cores=8) as tc:
        with tc.tile_pool(name="const", bufs=1) as cpool, \
             tc.tile_pool(name="work", bufs=2) as wpool, \
             tc.tile_pool(name="psum", bufs=2, space="PSUM") as ppool, \
             tc.tile_pool(name="psum1", bufs=1, space="PSUM") as ppool1:
            # ---- resident constants / indices ----
            def load(name, shape, d):
                t = cpool.tile(shape, d, tag=name)
                nc.sync.dma_start(t[:], inp[name][:])
                return t
            idxA = load("idxA", [128, SA // 16], dt.int16)
            idxB = load("idxB", [128, SB // 16], dt.int16)
            dsti = load("dsti", [128, ST // 16], dt.int16)
            doff = load("doff", [128, ST // 128], dt.float32)
            iota = load("iota128", [128, 128], dt.float32)
            ident = load("ident", [128, 128], dt.float32)
            RHSs = [load("RHS1", [48, 264], dt.float32)]
            for l, w in ((2, 260), (3, 130)):
                t = cpool.tile([128, 2, w], dt.float32, tag=f"RHS{l}")
                nc.sync.dma_start(
                    t[:], inp[f"RHS{l}"][:].rearrange("(k p) w -> p k w", p=128))
                RHSs.append(t)

            # ---- prologue: x0 = [emb | x], T1 rows ----
            embg = wpool.tile([128, NB, 64], dt.float32, tag="embg")
            embt = load("embt_idx", [128, SLICE // 16], dt.int16)
            gat(embg, inp["emb_pad"][:], embt[:], 0, NB, 64)
            for b in range(NB):
                xb = wpool.tile([128, 33], dt.float32, tag="xb")
                nc.sync.dma_start(xb[:], inp["x_rows"][b * 128:(b + 1) * 128, :])
                x0 = wpool.tile([128, 48], dt.float32, tag="x0")
                nc.vector.tensor_copy(x0[:, 0:EMB], embg[:, b, 0:EMB])
                nc.vector.tensor_copy(x0[:, EMB:48], xb[:, 1:33])
                pt = ppool1.tile([128, 128], dt.float32, tag="pt")
                nc.tensor.transpose(pt[0:48, :], x0[:], ident[:])
                x0T = wpool.tile([48, 128], dt.float32, tag="x0T")
                nc.vector.tensor_copy(x0T[:], pt[0:48, :])
                pn = ppool1.tile([128, 264], dt.float32, tag="pn")
                nc.tensor.matmul(pn[:, 0:264], x0T[:], RHSs[0][:], start=True, stop=True)
                row = wpool.tile([128, 384], dt.bfloat16, tag="row")
                nc.vector.tensor_copy(row[:, 0:264], pn[:, 0:264])
                nc.vector.memset(row[:, 264:384], 0)
                nc.sync.dma_start(sl_t[0][b * 128:(b + 1) * 128, :], row[:, 0:384])
            nc.gpsimd.collective_compute(
                "AllGather", mybir.AluOpType.bypass,
                ins=[sl_t[0][:]], outs=[T_t[0][:]],
                replica_groups=[list(range(8))])

            # ---- three GAT layers ----
            for li, (Din, HD, H, D, STRIDE, OFF) in enumerate(LCFG):
                RW = HD + H
                pa = pb = pt_ = 0
                for b in range(NB):
                    cA, cB = CA[b], CB[b]
                    C = cA + cB
                    G = wpool.tile([128, C, STRIDE], dt.bfloat16, tag="G")
                    gat(G, T_t[li][:, :], idxA[:], pa // 16, cA, STRIDE)
                    if cB:
                        gat(G[:, cA:C, :].rearrange("p c e -> p c e"),
                            T_t[li][HALF:, :], idxB[:], pb // 16, cB, STRIDE)
                    GD = wpool.tile([128, C, 128], dt.bfloat16, tag="GD")
                    gat(GD, sl_t[li][:, OFF:OFF + 128], dsti[:], pt_ // 16, C,
                        128, elem_step=STRIDE)
                    S = wpool.tile([128, C, 128], dt.bfloat16, tag="S")
                    nc.vector.tensor_tensor(
                        S[:],
                        doff[:, pt_ // 128: pt_ // 128 + C].unsqueeze(-1)
                            .broadcast_to([128, C, 128]),
                        iota[:].unsqueeze(1).broadcast_to([128, C, 128]),
                        A.is_equal)
                    lg = wpool.tile([128, C, H], dt.float32, tag="lg")
                    nc.vector.tensor_tensor(
                        lg[:], G[:, :, OFF:OFF + H], GD[:, :, H:2 * H], A.add)
                    lg2 = wpool.tile([128, C, H], dt.float32, tag="lg2")
                    nc.vector.tensor_scalar_mul(lg2[:], lg[:], 0.2)
                    nc.vector.tensor_tensor(lg[:], lg[:], lg2[:], A.max)
                    RT = wpool.tile([128, C, RW], dt.bfloat16, tag="RT")
                    nc.scalar.activation(RT[:, :, HD:HD + H], lg[:], F.Exp)
                    nc.vector.tensor_tensor(
                        RT[:, :, 0:HD].rearrange("p c (h d) -> p c h d", h=H),
                        G[:, :, 0:HD].rearrange("p c (h d) -> p c h d", h=H),
                        RT[:, :, HD:HD + H].unsqueeze(-1)
                            .broadcast_to([128, C, H, D]),
                        A.mult)
                    pe = ppool.tile([128, RW], dt.float32, tag="pe")
                    for ch in range(C):
                        nc.tensor.matmul(pe[:, 0:RW], S[:, ch, :], RT[:, ch, :],
                                         start=(ch == 0), stop=(ch == C - 1))
                    pa += cA * 128
                    pb += cB * 128
                    pt_ += C * 128
                    # ---- finalize + node phase ----
                    den = wpool.tile([128, H], dt.float32, tag="den")
                    nc.vector.tensor_scalar_add(den[:], pe[:, HD:HD + H], 1e-16)
                    rec = wpool.tile([128, H], dt.float32, tag="rec")
                    nc.vector.reciprocal(rec[:], den[:])
                    xo = wpool.tile([128, HD], dt.float32, tag="xo")
                    nc.vector.tensor_tensor(
                        xo[:].rearrange("p (h d) -> p h d", h=H),
                        pe[:, 0:HD].rearrange("p (h d) -> p h d", h=H),
                        rec[:].unsqueeze(-1).broadcast_to([128, H, D]),
                        A.mult)
                    if li < 2:
                        m = wpool.tile([128, HD], dt.float32, tag="melu")
                        nc.vector.tensor_scalar_min(m[:], xo[:], 0.0)
                        e1 = wpool.tile([128, HD], dt.float32, tag="e1")
                        nc.scalar.activation(e1[:], m[:], F.Exp)
                        nc.vector.tensor_scalar_max(xo[:], xo[:], 0.0)
                        nc.vector.tensor_tensor(xo[:], xo[:], e1[:], A.add)
                        nc.vector.tensor_scalar_add(xo[:], xo[:], -1.0)
                        # node phase: T_{l+1} row = [x @ W' | folds]
                        NDin, NHD, NH, ND, NSTRIDE, _ = LCFG[li + 1]
                        NW = NHD + 2 * NH
                        xT = wpool.tile([128, 2, 128], dt.float32, tag="xT")
                        for kc in range(2):
                            ptp = ppool1.tile([128, 128], dt.float32, tag="pt")
                            nc.tensor.transpose(
                                ptp[:], xo[:, kc * 128:(kc + 1) * 128], ident[:])
                            nc.vector.tensor_copy(xT[:, kc, :], ptp[:])
                        pn = ppool1.tile([128, 264], dt.float32, tag="pn")
                        for kc in range(2):
                            nc.tensor.matmul(pn[:, 0:NW], xT[:, kc, :],
                                             RHSs[li + 1][:, kc, :],
                                             start=(kc == 0), stop=(kc == 1))
                        row = wpool.tile([128, NSTRIDE], dt.bfloat16, tag="row")
                        nc.vector.tensor_copy(row[:, 0:NW], pn[:, 0:NW])
                        if NW < NSTRIDE:
                            nc.vector.memset(row[:, NW:NSTRIDE], 0)
                        nc.sync.dma_start(
                            sl_t[li + 1][b * 128:(b + 1) * 128, :],
                            row[:, 0:NSTRIDE])
                    else:
                        rowd = wpool.tile([128, TDEC_W], dt.bfloat16, tag="rowd")
                        nc.vector.tensor_copy(rowd[:], xo[:])
                        nc.sync.dma_start(
                            sl_d[b * 128:(b + 1) * 128, :], rowd[:])
                if li < 2:
                    nc.gpsimd.collective_compute(
                        "AllGather", mybir.AluOpType.bypass,
                        ins=[sl_t[li + 1][:]], outs=[T_t[li + 1][:]],
                        replica_groups=[list(range(8))])
                else:
                    nc.gpsimd.collective_compute(
                        "AllGather", mybir.AluOpType.bypass,
                        ins=[sl_d[:]], outs=[T_d[:]],
                        replica_groups=[list(range(8))])

            # ---- decoder ----
            iota384 = load("iota384", [128, TB_W], dt.float32)
            lsi = load("ls_idx", [128, SL // 16], dt.int16)
            ldi = load("ld_idx", [128, SL // 16], dt.int16)
            tlsi = load("tls_idx", [128, SL // 16], dt.int16)
            tld = load("tld", [128, SL // 128], dt.float32)
            W1a = load("Wl1a", [128, 64], dt.bfloat16)
            W1b = load("Wl1b", [128, 64], dt.bfloat16)
            W2d = load("Wl2", [64, 1], dt.bfloat16)
            bl1 = load("bl1", [64, 1], dt.float32)
            score_sb = cpool.tile([128, SL // 128], dt.float32, tag="score")
            pos = 0
            for gi in range(4):
                gls, gld = (HALF if gi >= 2 else 0), (HALF if gi % 2 else 0)
                for t0 in range(pos, pos + cfg["GSZ"][gi], DEC_TILE):
                    zl = wpool.tile([128, 1, DEC_TILE], dt.bfloat16, tag="zl")
                    nc.gpsimd.dma_gather(
                        zl[:], T_d[gls:, :], lsi[:, t0 // 16:(t0 + DEC_TILE) // 16],
                        DEC_TILE, DEC_TILE, TDEC_W, transpose=True)
                    zr = wpool.tile([128, 1, DEC_TILE], dt.bfloat16, tag="zr")
                    nc.gpsimd.dma_gather(
                        zr[:], T_d[gld:, :], ldi[:, t0 // 16:(t0 + DEC_TILE) // 16],
                        DEC_TILE, DEC_TILE, TDEC_W, transpose=True)
                    ph = ppool.tile([64, DEC_TILE], dt.float32, tag="ph")
                    nc.tensor.matmul(ph[:], W1a[:], zl[:, 0, :], start=True, stop=False)
                    nc.tensor.matmul(ph[:], W1b[:], zr[:, 0, :], start=False, stop=True)
                    hd = wpool.tile([64, DEC_TILE], dt.bfloat16, tag="hd")
                    nc.scalar.activation(hd[:], ph[:], F.Relu, bias=bl1[:])
                    TBg = wpool.tile([128, 4, TB_W], dt.bfloat16, tag="TBg")
                    nc.gpsimd.dma_gather(
                        TBg[:], inp["TBpad"][:], tlsi[:, t0 // 16:(t0 + DEC_TILE) // 16],
                        DEC_TILE, DEC_TILE, TB_W)
                    oh = wpool.tile([128, 4, TB_W], dt.bfloat16, tag="oh")
                    nc.vector.tensor_tensor(
                        oh[:],
                        tld[:, t0 // 128: t0 // 128 + 4].unsqueeze(-1)
                            .broadcast_to([128, 4, TB_W]),
                        iota384[:].unsqueeze(1).broadcast_to([128, 4, TB_W]),
                        A.is_equal)
                    tb2 = wpool.tile([128, 4, TB_W], dt.float32, tag="tb2")
                    nc.vector.tensor_tensor(tb2[:], TBg[:], oh[:], A.mult)
                    bias = wpool.tile([128, 4], dt.float32, tag="bias")
                    nc.vector.tensor_reduce(bias[:], tb2[:],
                                            mybir.AxisListType.X, A.add)
                    for sub in range(4):
                        pss = ppool1.tile([128, 1], dt.float32, tag="pss")
                        nc.tensor.matmul(pss[:], hd[:, sub * 128:(sub + 1) * 128],
                                         W2d[:], start=True, stop=True)
                        col = t0 // 128 + sub
                        nc.vector.tensor_tensor(
                            score_sb[:, col:col + 1], pss[:],
                            bias[:, sub:sub + 1], A.add)
                pos += cfg["GSZ"][gi]
            if cfg["bl2"] != 0.0:
                nc.vector.tensor_scalar_add(score_sb[:], score_sb[:], cfg["bl2"])
            nc.sync.dma_start(
                score_out[:].rearrange("(c p) o -> p (c o)", p=128), score_sb[:])
    nc.finalize()
    return nc


def kernel(**inputs):
    inputs = {k: np.asarray(v) for k, v in inputs.items()}
    in_maps, cfg = prep(**inputs)
    nc = build(cfg)
    from concourse.bass_utils import run_bass_kernel_spmd
    res = run_bass_kernel_spmd(nc, in_maps, core_ids=list(range(8)))
    out = np.zeros((EL, 1), np.float32)
    for c in range(8):
        sc = res.results[c]["score"][:, 0]
        m = cfg["slotmap"][c] >= 0
        out[cfg["slotmap"][c][m], 0] = sc[m]
    return out



# revision 5
# speedup vs baseline: 2.1664x; 2.1664x over previous
"""Trainium2 Bass kernel for nn_DAGLinkPredictor (3-layer GAT + edge decoder).

V2. Sharding: dst-node-sharded GAT across 8 cores. Edges (incl self loops)
sorted by dst, grouped into per-core 128-node dst blocks.

Key structure (vs v1):
  - Gather tables hold h only (256/256/128 bf16 elems = 512/512/256B rows,
    the dma_gather 256B-granularity minimum). als = h@a_src is computed
    on-device from the gathered rows (DVE mult + reduce); ald comes from the
    block-local node rows via a transposed-one-hot matmul (PE). No second
    (dst-side) gather at all.
  - Tables are split in two halves (A: slice rows < 3200, B: rest), which
    both keeps gather indices < 32768 (int16) and lets each half's AllGather
    start mid node-phase (A issued after block 24, B after block 48),
    overlapping the collective with compute.
  - Gathers read the Shared AllGather outputs directly (measured as fast as
    Internal DRAM at these row sizes).
  - Softmax is denominator-style (exactly segment softmax, no segment-max
    needed at these logit scales).
  - Decoder: trans_bias gathers+reduction run before the TD-dependent
    z-gathers so they don't stall behind the final collectives.
"""
import numpy as np
import ml_dtypes

N = 50000
NP = 50176            # padded nodes: 8 * 6272
SLICE = NP // 8       # 6272 nodes per core
NB = SLICE // 128     # 49 blocks per core
HSLA = 3200           # A-half rows per slice (25 blocks)
HSLB = SLICE - HSLA   # B-half rows per slice (24 blocks)
NBA = HSLA // 128     # 25
NROWA = 8 * HSLA      # 25600 rows in table A
NROWB = 8 * HSLB      # 24576 rows in table B
E = 800000
EL = 100000
NTYPES = 311
EMB = 16
COMB = 48

# per-layer: (Din, HD, H, D); table width for layer l's edge phase = HD
LCFG = [
    (48, 256, 4, 64),
    (256, 256, 2, 128),
    (256, 128, 1, 128),
]
TDEC_W = 128          # decoder table row elems
TB_W = 384            # padded trans_bias row
DEC_TILE = 512
GMAX = 6              # max 128-chunks per dma_gather (descriptor-ring bound)

bf16 = ml_dtypes.bfloat16


def _wrap_idx(vals):
    """int16 index array for dma_gather: [128, n/16], wrapped over 16
    partitions and replicated across the 8 gpsimd cores."""
    n = len(vals)
    assert n % 16 == 0
    a = np.zeros((128, n // 16), np.int16)
    v = np.asarray(vals, np.int64)
    assert v.min() >= 0 and v.max() < 32768, (v.min(), v.max())
    w = v.reshape(n // 16, 16).T.astype(np.int16)  # [16, n/16]
    for g in range(8):
        a[16 * g:16 * g + 16, :] = w
    return a


def _slotmajor(vals, dtype):
    """[128, n/128] array with element (p, c) = vals[c*128+p]."""
    n = len(vals)
    assert n % 128 == 0
    return np.asarray(vals, np.float64).reshape(n // 128, 128).T.astype(dtype)


def prep(x, edge_index, edge_label_index, emb, W1, a_src1, a_dst1, b1,
         W2, a_src2, a_dst2, b2, W3, a_src3, a_dst3, b3,
         Wl1, bl1, Wl2, bl2, trans_bias):
    """Host-side (integer/index + weight-layout) preprocessing."""
    types = x[:, 0].astype(np.int64)

    # --- weight folds: RHS_l = [W_l | W_l@a_dst per head] ---
    def fold(W, a_d, H, D):
        cols_d = np.stack([W[:, h * D:(h + 1) * D] @ a_d[h] for h in range(H)], 1)
        return np.concatenate([W, cols_d], 1).astype(np.float32)
    RHS = [fold(W1, a_dst1, 4, 64),
           fold(W2, a_dst2, 2, 128),
           fold(W3, a_dst3, 1, 128)]
    ASRC = [np.tile(a.reshape(1, -1), (128, 1)).astype(bf16)
            for a in (a_src1, a_src2, a_src3)]

    emb_pad = np.zeros((NTYPES, 64), np.float32)
    emb_pad[:, :EMB] = emb
    TBpad = np.zeros((NTYPES, TB_W), bf16)
    TBpad[:, :NTYPES] = trans_bias.astype(bf16)

    # --- edges: add self loops, sort by dst, bucket per core / block ---
    loops = np.arange(N, dtype=np.int64)
    src = np.concatenate([edge_index[0].astype(np.int64), loops])
    dst = np.concatenate([edge_index[1].astype(np.int64), loops])
    order = np.argsort(dst, kind="stable")
    src, dst = src[order], dst[order]

    sown = src // SLICE
    srem = src % SLICE
    inA = srem < HSLA
    rowA = sown * HSLA + srem            # valid where inA
    rowB = sown * HSLB + (srem - HSLA)   # valid where ~inA

    blk = dst // 128          # global dst block id (0..391)
    per = [[None] * NB for _ in range(8)]
    for c in range(8):
        for b in range(NB):
            m = blk == c * NB + b
            per[c][b] = (rowA[m & inA], dst[m & inA],
                         rowB[m & ~inA], dst[m & ~inA])
    CA = [max(1, max((len(per[c][b][0]) + 127) // 128 for c in range(8)))
          for b in range(NB)]
    CB = [max((len(per[c][b][2]) + 127) // 128 for c in range(8))
          for b in range(NB)]

    idxA, idxB, doff = [], [], []
    for c in range(8):
        la, lb, lo = [], [], []
        for b in range(NB):
            base = (c * NB + b) * 128
            for half, cnt in ((0, CA[b]), (1, CB[b])):
                rows = per[c][b][0 + 2 * half]
                dsts = per[c][b][1 + 2 * half]
                ns = cnt * 128
                sp = np.zeros(ns, np.int64)
                sp[:len(rows)] = rows
                (la if half == 0 else lb).append(sp)
                off = np.full(ns, 255, np.int64)
                off[:len(dsts)] = dsts - base
                lo.append(off)
        idxA.append(_wrap_idx(np.concatenate(la)))
        idxB.append(_wrap_idx(np.concatenate(lb)))
        doff.append(_slotmajor(np.concatenate(lo), bf16))

    # --- label edges: 4 groups by (ls-half, ld-half), padded per group ---
    ls = edge_label_index[0].astype(np.int64)
    ld_ = edge_label_index[1].astype(np.int64)
    elpc = (EL + 7) // 8
    groups_sz = np.zeros((8, 4), np.int64)
    per_dec = [[None] * 4 for _ in range(8)]
    lsA = (ls % SLICE) < HSLA
    ldA = (ld_ % SLICE) < HSLA
    lsrow = np.where(lsA, (ls // SLICE) * HSLA + ls % SLICE,
                     (ls // SLICE) * HSLB + ls % SLICE - HSLA)
    ldrow = np.where(ldA, (ld_ // SLICE) * HSLA + ld_ % SLICE,
                     (ld_ // SLICE) * HSLB + ld_ % SLICE - HSLA)
    for c in range(8):
        lo_, hi_ = c * elpc, min((c + 1) * elpc, EL)
        eidx = np.arange(lo_, hi_)
        g = (~lsA[eidx]).astype(np.int64) * 2 + (~ldA[eidx])
        for gi in range(4):
            per_dec[c][gi] = eidx[g == gi]
            groups_sz[c, gi] = len(per_dec[c][gi])
    GSZ = [int(-(-groups_sz[:, gi].max() // DEC_TILE) * DEC_TILE)
           for gi in range(4)]
    SL = sum(GSZ)
    lsw, ldw, tlsw, tldw, slotmap = [], [], [], [], []
    for c in range(8):
        a_ls = np.zeros(SL, np.int64)
        a_ld = np.zeros(SL, np.int64)
        a_tls = np.zeros(SL, np.int64)
        a_tld = np.zeros(SL, np.int64)
        smap = np.full(SL, -1, np.int64)
        pos = 0
        for gi in range(4):
            e = per_dec[c][gi]
            n = len(e)
            a_ls[pos:pos + n] = lsrow[e]
            a_ld[pos:pos + n] = ldrow[e]
            a_tls[pos:pos + n] = types[np.minimum(ls[e], N - 1)]
            a_tld[pos:pos + n] = types[np.minimum(ld_[e], N - 1)]
            smap[pos:pos + n] = e
            pos += GSZ[gi]
        lsw.append(_wrap_idx(a_ls))
        ldw.append(_wrap_idx(a_ld))
        tlsw.append(_wrap_idx(a_tls))
        tldw.append(_slotmajor(a_tld, np.float32))
        slotmap.append(smap)

    iota128 = np.tile(np.arange(128, dtype=np.float64)[None, :],
                      (128, 1)).astype(bf16)
    iota384 = np.tile(np.arange(TB_W, dtype=np.float32)[None, :], (128, 1))
    ident = np.eye(128, dtype=np.float32)
    identb = np.eye(128, dtype=bf16)

    in_maps = []
    for c in range(8):
        xr = np.zeros((SLICE, 33), np.float32)
        n0 = c * SLICE
        n1 = min((c + 1) * SLICE, N)
        if n1 > n0:
            xr[:n1 - n0] = x[n0:n1]
        ti = np.zeros(SLICE, np.int64)
        if n1 > n0:
            ti[:n1 - n0] = types[n0:n1]
        in_maps.append(dict(
            x_rows=xr, embt_idx=_wrap_idx(ti),
            emb_pad=emb_pad,
            RHS1=RHS[0], RHS2=RHS[1], RHS3=RHS[2],
            asrc1=ASRC[0], asrc2=ASRC[1], asrc3=ASRC[2],
            idxA=idxA[c], idxB=idxB[c], doff=doff[c],
            ls_idx=lsw[c], ld_idx=ldw[c], tls_idx=tlsw[c], tld=tldw[c],
            iota128=iota128, iota384=iota384, ident=ident, identb=identb,
            TBpad=TBpad,
            Wl1a=Wl1[:128].astype(bf16), Wl1b=Wl1[128:].astype(bf16),
            Wl2=Wl2.astype(bf16), bl1=bl1.reshape(64, 1).astype(np.float32),
        ))
    cfg = dict(CA=CA, CB=CB, GSZ=GSZ, SL=SL,
               SA=sum(CA) * 128, SB=sum(CB) * 128,
               ST=(sum(CA) + sum(CB)) * 128,
               bl2=float(np.asarray(bl2).reshape(-1)[0]),
               b=[np.asarray(b1), np.asarray(b2), np.asarray(b3)],
               slotmap=slotmap)
    return in_maps, cfg


# ---------------------------------------------------------------- golden ---
def golden(in_maps, cfg):
    """numpy mirror of the device algorithm (fp32; layout-accurate)."""
    CA, CB = cfg["CA"], cfg["CB"]
    out_all = None
    # tables per layer: TA [NROWA, W], TB [NROWB, W]
    PREV = None
    for li, (Din, HD, H, D) in enumerate(LCFG):
        TA = np.zeros((NROWA, HD), np.float32)
        TB = np.zeros((NROWB, HD), np.float32)
        ALD = []   # per core [SLICE, H]
        for c in range(8):
            im = in_maps[c]
            if li == 0:
                embg = im["emb_pad"][_unwrap(im["embt_idx"], SLICE)][:, :EMB]
                x0 = np.concatenate([embg, im["x_rows"][:, 1:33]], 1)
                pn = x0 @ im["RHS1"]
            else:
                pn = PREV[c] @ im[f"RHS{li + 1}"]
            h = pn[:, :HD].astype(bf16).astype(np.float32)
            ald = pn[:, HD:HD + H].astype(bf16).astype(np.float32)
            TA[c * HSLA:(c + 1) * HSLA] = h[:HSLA]
            TB[c * HSLB:(c + 1) * HSLB] = h[HSLA:]
            ALD.append(ald)
        PREV = []
        for c in range(8):
            im = in_maps[c]
            ia = _unwrap(im["idxA"], cfg["SA"])
            ib = _unwrap(im["idxB"], cfg["SB"])
            dof = im["doff"].astype(np.float64).T.reshape(-1)  # slot order
            asrc = im[f"asrc{li + 1}"][0].astype(np.float32)   # [HD]
            xl = np.zeros((SLICE, HD), np.float32)
            pa = pb = pt = 0
            for b in range(NB):
                sA, sB = CA[b] * 128, CB[b] * 128
                nsl = sA + sB
                G = np.concatenate([TA[ia[pa:pa + sA]], TB[ib[pb:pb + sB]]])
                pa += sA; pb += sB
                off = dof[pt:pt + nsl].astype(np.int64)
                pt += nsl
                S = (off[:, None] == np.arange(128)[None, :])
                amul = (G * asrc[None, :]).astype(bf16).astype(np.float32)
                als = amul.reshape(nsl, H, D).sum(2)              # [nsl,H]
                aldb = ALD[c][b * 128:(b + 1) * 128]              # [128,H]
                ald_slot = S.astype(np.float32) @ aldb            # [nsl,H]
                u = als + ald_slot
                lg = np.where(u > 0, u, 0.2 * u)
                e = np.exp(lg).astype(bf16).astype(np.float32)
                msg = (G.reshape(nsl, H, D) * e[:, :, None]
                       ).reshape(nsl, HD).astype(bf16).astype(np.float32)
                num = S.T.astype(np.float32) @ msg
                den = S.T.astype(np.float32) @ e
                r = 1.0 / (den + 1e-16)
                xb = (num.reshape(128, H, D) * r[:, :, None]).reshape(128, HD)
                bvec = cfg["b"][li]
                if np.any(bvec != 0):
                    xb = xb + bvec
                if li < 2:
                    xb = np.maximum(xb, 0) + np.exp(np.minimum(xb, 0)) - 1
                xl[b * 128:(b + 1) * 128] = xb
            PREV.append(xl)
    # decode tables
    TDA = np.zeros((NROWA, TDEC_W), np.float32)
    TDB = np.zeros((NROWB, TDEC_W), np.float32)
    for c in range(8):
        z = PREV[c].astype(bf16).astype(np.float32)
        TDA[c * HSLA:(c + 1) * HSLA] = z[:HSLA]
        TDB[c * HSLB:(c + 1) * HSLB] = z[HSLA:]
    scores = []
    for c in range(8):
        im = in_maps[c]
        lsv = _unwrap(im["ls_idx"], cfg["SL"])
        ldv = _unwrap(im["ld_idx"], cfg["SL"])
        tlsv = _unwrap(im["tls_idx"], cfg["SL"])
        tldv = im["tld"].T.reshape(-1)
        zl = np.zeros((cfg["SL"], TDEC_W), np.float32)
        zr = np.zeros((cfg["SL"], TDEC_W), np.float32)
        pos = 0
        for gi in range(4):
            sl_ = slice(pos, pos + cfg["GSZ"][gi])
            zl[sl_] = (TDB if gi >= 2 else TDA)[lsv[sl_]]
            zr[sl_] = (TDB if gi % 2 else TDA)[ldv[sl_]]
            pos += cfg["GSZ"][gi]
        W1a = im["Wl1a"].astype(np.float32)
        W1b = im["Wl1b"].astype(np.float32)
        h = np.maximum(zl @ W1a + zr @ W1b + im["bl1"].T, 0
                       ).astype(bf16).astype(np.float32)
        base = h @ im["Wl2"].astype(np.float32) + cfg["bl2"]
        TBg = im["TBpad"].astype(np.float32)[tlsv]
        oh = (tldv[:, None] == np.arange(TB_W)[None, :])
        bias = (TBg * oh).sum(1)
        scores.append(base[:, 0] + bias)
    out = np.zeros((EL, 1), np.float32)
    for c in range(8):
        m = cfg["slotmap"][c] >= 0
        out[cfg["slotmap"][c][m], 0] = scores[c][m]
    return out


def _unwrap(w, n):
    return w[:16, :].T.reshape(-1)[:n].astype(np.int64)


# ----------------------------------------------------------------- device ---
def build(cfg):
    import concourse.bacc as bacc
    import concourse.mybir as mybir
    from concourse.tile import TileContext
    dt = mybir.dt
    F = mybir.ActivationFunctionType
    A = mybir.AluOpType
    CA, CB, SL = cfg["CA"], cfg["CB"], cfg["SL"]
    SA, SB, ST = cfg["SA"], cfg["SB"], cfg["ST"]
    TBLW = [256, 256, 128]

    nc = bacc.Bacc(num_devices=8, dynamic_dma_scratch_size=32768)

    def gat(out_ap, in_ap, idx_tile, col0, nchunk, elem, **kw):
        for s0 in range(0, nchunk, GMAX):
            s1 = min(s0 + GMAX, nchunk)
            nc.gpsimd.dma_gather(
                out_ap[:, s0:s1, :], in_ap,
                idx_tile[:, col0 + s0 * 8: col0 + s1 * 8],
                (s1 - s0) * 128, (s1 - s0) * 128, elem, **kw)

    inp = {}
    for name, shape, d in [
        ("x_rows", [SLICE, 33], dt.float32),
        ("embt_idx", [128, SLICE // 16], dt.int16),
        ("emb_pad", [NTYPES, 64], dt.float32),
        ("RHS1", [48, 260], dt.float32),
        ("RHS2", [256, 258], dt.float32),
        ("RHS3", [256, 129], dt.float32),
        ("asrc1", [128, 256], dt.bfloat16),
        ("asrc2", [128, 256], dt.bfloat16),
        ("asrc3", [128, 128], dt.bfloat16),
        ("idxA", [128, SA // 16], dt.int16),
        ("idxB", [128, SB // 16], dt.int16),
        ("doff", [128, ST // 128], dt.bfloat16),
        ("ls_idx", [128, SL // 16], dt.int16),
        ("ld_idx", [128, SL // 16], dt.int16),
        ("tls_idx", [128, SL // 16], dt.int16),
        ("tld", [128, SL // 128], dt.float32),
        ("iota128", [128, 128], dt.bfloat16),
        ("iota384", [128, TB_W], dt.float32),
        ("ident", [128, 128], dt.float32),
        ("identb", [128, 128], dt.bfloat16),
        ("TBpad", [NTYPES, TB_W], dt.bfloat16),
        ("Wl1a", [128, 64], dt.bfloat16),
        ("Wl1b", [128, 64], dt.bfloat16),
        ("Wl2", [64, 1], dt.bfloat16),
        ("bl1", [64, 1], dt.float32),
    ]:
        inp[name] = nc.dram_tensor(name, shape, d, kind="ExternalInput")
    score_out = nc.dram_tensor("score", [SL, 1], dt.float32, kind="ExternalOutput")

    slA = [nc.dram_tensor(f"slA{l}", [HSLA, TBLW[l]], dt.bfloat16,
                          kind="Internal") for l in range(3)]
    slB = [nc.dram_tensor(f"slB{l}", [HSLB, TBLW[l]], dt.bfloat16,
                          kind="Internal") for l in range(3)]
    slDA = nc.dram_tensor("slDA", [HSLA, TDEC_W], dt.bfloat16, kind="Internal")
    slDB = nc.dram_tensor("slDB", [HSLB, TDEC_W], dt.bfloat16, kind="Internal")
    T_A = [nc.dram_tensor(f"TA{l}", [NROWA, TBLW[l]], dt.bfloat16,
                          kind="Internal", addr_space="Shared") for l in range(3)]
    T_B = [nc.dram_tensor(f"TB{l}", [NROWB, TBLW[l]], dt.bfloat16,
                          kind="Internal", addr_space="Shared") for l in range(3)]
    TD_A = nc.dram_tensor("TDA", [NROWA, TDEC_W], dt.bfloat16,
                          kind="Internal", addr_space="Shared")
    TD_B = nc.dram_tensor("TDB", [NROWB, TDEC_W], dt.bfloat16,
                          kind="Internal", addr_space="Shared")

    def allgather(src, dst):
        nc.gpsimd.collective_compute(
            "AllGather", mybir.AluOpType.bypass,
            ins=[src[:]], outs=[dst[:]],
            replica_groups=[list(range(8))])

    with TileContext(nc, num_cores=8) as tc:
        with tc.tile_pool(name="const", bufs=1) as cpool:
            def load(name, shape, d):
                t = cpool.tile(shape, d, tag=name)
                nc.sync.dma_start(t[:], inp[name][:])
                return t
            idxA = load("idxA", [128, SA // 16], dt.int16)
            idxB = load("idxB", [128, SB // 16], dt.int16)
            doff = load("doff", [128, ST // 128], dt.bfloat16)
            iota = load("iota128", [128, 128], dt.bfloat16)
            ident = load("ident", [128, 128], dt.float32)
            identb = load("identb", [128, 128], dt.bfloat16)
            RHSs = [load("RHS1", [48, 260], dt.float32)]
            for l, w in ((2, 258), (3, 129)):
                t = cpool.tile([128, 2, w], dt.float32, tag=f"RHS{l}")
                nc.sync.dma_start(
                    t[:], inp[f"RHS{l}"][:].rearrange("(k p) w -> p k w", p=128))
                RHSs.append(t)
            asrc = [load("asrc1", [128, 256], dt.bfloat16),
                    load("asrc2", [128, 256], dt.bfloat16),
                    load("asrc3", [128, 128], dt.bfloat16)]
            aldres = [cpool.tile([128, NB, h], dt.bfloat16, tag=f"aldres{l}",
                                 name=f"aldres{l}")
                      for l, h in ((0, 4), (1, 2), (2, 1))]

            wpool = tc.alloc_tile_pool(name="work", bufs=2)
            ppool = tc.alloc_tile_pool(name="psum", bufs=2, space="PSUM")
            ppool1 = tc.alloc_tile_pool(name="psum1", bufs=1, space="PSUM")

            def row_out(b, pn_ap, li_next, HDn, Hn):
                """copy node-phase PSUM to bf16 row, DMA to slA/slB, stash ald."""
                if li_next < 3:
                    row = wpool.tile([128, HDn], dt.bfloat16, tag="row")
                    nc.scalar.activation(row[:], pn_ap[:, 0:HDn], F.Copy)
                    nc.vector.tensor_copy(aldres[li_next][:, b, :],
                                          pn_ap[:, HDn:HDn + Hn])
                    dstA, dstB = slA[li_next], slB[li_next]
                else:
                    row = wpool.tile([128, TDEC_W], dt.bfloat16, tag="row")
                    nc.scalar.activation(row[:], pn_ap[:], F.Copy)
                    dstA, dstB = slDA, slDB
                if b < NBA:
                    nc.sync.dma_start(dstA[b * 128:(b + 1) * 128, :], row[:])
                else:
                    b2 = b - NBA
                    nc.sync.dma_start(dstB[b2 * 128:(b2 + 1) * 128, :], row[:])

            # ---- prologue: x0 = [emb | x], layer-1 table rows ----
            embg = cpool.tile([128, NB, 64], dt.float32, tag="embg")
            embt = load("embt_idx", [128, SLICE // 16], dt.int16)
            gat(embg, inp["emb_pad"][:], embt[:], 0, NB, 64)
            for b in range(NB):
                xb = wpool.tile([128, 33], dt.float32, tag="xb")
                nc.sync.dma_start(xb[:], inp["x_rows"][b * 128:(b + 1) * 128, :])
                x0 = wpool.tile([128, 48], dt.float32, tag="x0")
                nc.vector.tensor_copy(x0[:, 0:EMB], embg[:, b, 0:EMB])
                nc.vector.tensor_copy(x0[:, EMB:48], xb[:, 1:33])
                pt = ppool1.tile([128, 128], dt.float32, tag="pt")
                nc.tensor.transpose(pt[0:48, :], x0[:], ident[:])
                x0T = wpool.tile([48, 128], dt.float32, tag="x0T")
                nc.scalar.activation(x0T[:], pt[0:48, :], F.Copy)
                pn = ppool1.tile([128, 260], dt.float32, tag="pn")
                nc.tensor.matmul(pn[:, 0:260], x0T[:], RHSs[0][:],
                                 start=True, stop=True)
                row_out(b, pn, 0, 256, 4)
                if b == NBA - 1:
                    allgather(slA[0], T_A[0])
            allgather(slB[0], T_B[0])

            # ---- three GAT layers ----
            for li, (Din, HD, H, D) in enumerate(LCFG):
                RW = HD + H
                pa = pb = pt_ = 0
                for b in range(NB):
                    cA, cB = CA[b], CB[b]
                    C = cA + cB
                    G = wpool.tile([128, C, HD], dt.bfloat16, tag="G")
                    gat(G, T_A[li][:], idxA[:], pa // 16, cA, HD)
                    if cB:
                        gat(G[:, cA:C, :], T_B[li][:], idxB[:], pb // 16, cB, HD)
                    # als = per-head <h, a_src>: mult into RT scratch, reduce
                    RT = wpool.tile([128, C, RW], dt.bfloat16, tag="RT")
                    nc.vector.tensor_tensor(
                        RT[:, :, 0:HD],
                        G[:],
                        asrc[li][:].unsqueeze(1).broadcast_to([128, C, HD]),
                        A.mult)
                    als = wpool.tile([128, C, H], dt.float32, tag="als")
                    nc.vector.tensor_reduce(
                        als[:],
                        RT[:, :, 0:HD].rearrange("p c (h d) -> p c h d", h=H),
                        mybir.AxisListType.X, A.add)
                    # one-hot S (slot -> dst offset)
                    S = wpool.tile([128, C, 128], dt.bfloat16, tag="S")
                    nc.vector.tensor_tensor(
                        S[:],
                        doff[:, pt_ // 128: pt_ // 128 + C].unsqueeze(-1)
                            .broadcast_to([128, C, 128]),
                        iota[:].unsqueeze(1).broadcast_to([128, C, 128]),
                        A.is_equal)
                    # ald per slot: transpose S chunks, matmul with ald block
                    st_all = wpool.tile([128, C, 128], dt.bfloat16, tag="st")
                    ups = ppool1.tile([128, C, H], dt.float32, tag="ups")
                    for ch in range(C):
                        stp = ppool.tile([128, 128], dt.bfloat16, tag="stp")
                        nc.tensor.transpose(stp[:], S[:, ch, :], identb[:])
                        nc.scalar.activation(st_all[:, ch, :], stp[:], F.Copy)
                        nc.tensor.matmul(ups[:, ch, :], st_all[:, ch, :],
                                         aldres[li][:, b, :],
                                         start=True, stop=True)
                    u = wpool.tile([128, C, H], dt.float32, tag="u")
                    nc.vector.tensor_tensor(u[:], als[:], ups[:], A.add)
                    nc.scalar.activation(u[:], u[:], F.Lrelu, alpha=0.2)
                    nc.scalar.activation(RT[:, :, HD:HD + H], u[:], F.Exp)
                    # messages = h * e
                    nc.vector.tensor_tensor(
                        RT[:, :, 0:HD].rearrange("p c (h d) -> p c h d", h=H),
                        G[:].rearrange("p c (h d) -> p c h d", h=H),
                        RT[:, :, HD:HD + H].unsqueeze(-1)
                            .broadcast_to([128, C, H, D]),
                        A.mult)
                    pe = ppool.tile([128, RW], dt.float32, tag="pe")
                    for ch in range(C):
                        nc.tensor.matmul(pe[:, 0:RW], S[:, ch, :], RT[:, ch, :],
                                         start=(ch == 0), stop=(ch == C - 1))
                    pa += cA * 128
                    pb += cB * 128
                    pt_ += C * 128
                    # ---- finalize + node phase ----
                    den = wpool.tile([128, H], dt.float32, tag="den")
                    nc.vector.tensor_scalar_add(den[:], pe[:, HD:HD + H], 1e-16)
                    rec = wpool.tile([128, H], dt.float32, tag="rec")
                    nc.vector.reciprocal(rec[:], den[:])
                    xo = wpool.tile([128, HD], dt.float32, tag="xo")
                    nc.vector.tensor_tensor(
                        xo[:].rearrange("p (h d) -> p h d", h=H),
                        pe[:, 0:HD].rearrange("p (h d) -> p h d", h=H),
                        rec[:].unsqueeze(-1).broadcast_to([128, H, D]),
                        A.mult)
                    if li < 2:
                        m = wpool.tile([128, HD], dt.float32, tag="melu")
                        nc.vector.tensor_scalar_min(m[:], xo[:], 0.0)
                        nc.scalar.activation(m[:], m[:], F.Exp)
                        nc.vector.tensor_scalar_max(xo[:], xo[:], 0.0)
                        nc.vector.tensor_tensor(xo[:], xo[:], m[:], A.add)
                        nc.vector.tensor_scalar_add(xo[:], xo[:], -1.0)
                        NHD, NH = LCFG[li + 1][1], LCFG[li + 1][2]
                        NW = NHD + NH
                        xT = wpool.tile([128, 2, 128], dt.float32, tag="xT")
                        for kc in range(2):
                            ptp = ppool1.tile([128, 128], dt.float32, tag="pt")
                            nc.tensor.transpose(
                                ptp[:], xo[:, kc * 128:(kc + 1) * 128], ident[:])
                            nc.scalar.activation(xT[:, kc, :], ptp[:], F.Copy)
                        pn = ppool1.tile([128, 260], dt.float32, tag="pn")
                        for kc in range(2):
                            nc.tensor.matmul(pn[:, 0:NW], xT[:, kc, :],
                                             RHSs[li + 1][:, kc, :],
                                             start=(kc == 0), stop=(kc == 1))
                        row_out(b, pn, li + 1, NHD, NH)
                    else:
                        row_out(b, xo, 3, TDEC_W, 0)
                    if b == NBA - 1:
                        if li < 2:
                            allgather(slA[li + 1], T_A[li + 1])
                        else:
                            allgather(slDA, TD_A)
                if li < 2:
                    allgather(slB[li + 1], T_B[li + 1])
                else:
                    allgather(slDB, TD_B)

            ppool1.release()
            ppool.release()
            wpool.release()
            wpool = tc.alloc_tile_pool(name="dwork", bufs=2)
            ppool = tc.alloc_tile_pool(name="dpsum", bufs=2, space="PSUM")

            # ---- decoder ----
            iota384 = load("iota384", [128, TB_W], dt.float32)
            lsi = load("ls_idx", [128, SL // 16], dt.int16)
            ldi = load("ld_idx", [128, SL // 16], dt.int16)
            tlsi = load("tls_idx", [128, SL // 16], dt.int16)
            tld = load("tld", [128, SL // 128], dt.float32)
            W1a = load("Wl1a", [128, 64], dt.bfloat16)
            W1b = load("Wl1b", [128, 64], dt.bfloat16)
            W2d = load("Wl2", [64, 1], dt.bfloat16)
            bl1 = load("bl1", [64, 1], dt.float32)
            score_sb = cpool.tile([128, SL // 128], dt.float32, tag="score")
            biases = cpool.tile([128, SL // 128], dt.float32, tag="biases")
            # pass 1: trans_bias (independent of the TD collectives)
            for t0 in range(0, SL, DEC_TILE):
                TBg = wpool.tile([128, 4, TB_W], dt.bfloat16, tag="TBg")
                nc.gpsimd.dma_gather(
                    TBg[:], inp["TBpad"][:],
                    tlsi[:, t0 // 16:(t0 + DEC_TILE) // 16],
                    DEC_TILE, DEC_TILE, TB_W)
                oh = wpool.tile([128, 4, TB_W], dt.bfloat16, tag="oh")
                nc.vector.tensor_tensor(
                    oh[:],
                    tld[:, t0 // 128: t0 // 128 + 4].unsqueeze(-1)
                        .broadcast_to([128, 4, TB_W]),
                    iota384[:].unsqueeze(1).broadcast_to([128, 4, TB_W]),
                    A.is_equal)
                tb2 = wpool.tile([128, 4, TB_W], dt.bfloat16, tag="tb2")
                nc.vector.tensor_tensor(tb2[:], TBg[:], oh[:], A.mult)
                nc.vector.tensor_reduce(
                    biases[:, t0 // 128: t0 // 128 + 4], tb2[:],
                    mybir.AxisListType.X, A.add)
            # pass 2: z gathers + MLP
            pos = 0
            for gi in range(4):
                Tsrc = TD_B if gi >= 2 else TD_A
                Tdst = TD_B if gi % 2 else TD_A
                for t0 in range(pos, pos + cfg["GSZ"][gi], DEC_TILE):
                    zl = wpool.tile([128, 1, DEC_TILE], dt.bfloat16, tag="zl")
                    nc.gpsimd.dma_gather(
                        zl[:], Tsrc[:], lsi[:, t0 // 16:(t0 + DEC_TILE) // 16],
                        DEC_TILE, DEC_TILE, TDEC_W, transpose=True)
                    zr = wpool.tile([128, 1, DEC_TILE], dt.bfloat16, tag="zr")
                    nc.gpsimd.dma_gather(
                        zr[:], Tdst[:], ldi[:, t0 // 16:(t0 + DEC_TILE) // 16],
                        DEC_TILE, DEC_TILE, TDEC_W, transpose=True)
                    ph = ppool.tile([64, DEC_TILE], dt.float32, tag="ph")
                    nc.tensor.matmul(ph[:], W1a[:], zl[:, 0, :],
                                     start=True, stop=False)
                    nc.tensor.matmul(ph[:], W1b[:], zr[:, 0, :],
                                     start=False, stop=True)
                    hd = wpool.tile([64, DEC_TILE], dt.bfloat16, tag="hd")
                    nc.scalar.activation(hd[:], ph[:], F.Relu, bias=bl1[:])
                    for sub in range(4):
                        pss = ppool.tile([128, 1], dt.float32, tag="pss")
                        nc.tensor.matmul(pss[:], hd[:, sub * 128:(sub + 1) * 128],
                                         W2d[:], start=True, stop=True)
                        col = t0 // 128 + sub
                        nc.vector.tensor_tensor(
                            score_sb[:, col:col + 1], pss[:],
                            biases[:, col:col + 1], A.add)
                pos += cfg["GSZ"][gi]
            if cfg["bl2"] != 0.0:
                nc.vector.tensor_scalar_add(score_sb[:], score_sb[:], cfg["bl2"])
            nc.sync.dma_start(
                score_out[:].rearrange("(c p) o -> p (c o)", p=128), score_sb[:])
            ppool.release()
            wpool.release()
    nc.finalize()
    return nc


def kernel(**inputs):
    inputs = {k: np.asarray(v) for k, v in inputs.items()}
    in_maps, cfg = prep(**inputs)
    nc = build(cfg)
    from concourse.bass_utils import run_bass_kernel_spmd
    res = run_bass_kernel_spmd(nc, in_maps, core_ids=list(range(8)))
    out = np.zeros((EL, 1), np.float32)
    for c in range(8):
        sc = res.results[c]["score"][:, 0]
        m = cfg["slotmap"][c] >= 0
        out[cfg["slotmap"][c][m], 0] = sc[m]
    return out


# revision 8
# speedup vs baseline: 2.6415x; 1.2193x over previous
"""Trainium2 Bass kernel for nn_DAGLinkPredictor (3-layer GAT + edge decoder).

V2. Sharding: dst-node-sharded GAT across 8 cores. Edges (incl self loops)
sorted by dst, grouped into per-core 128-node dst blocks.

Key structure (vs v1):
  - Gather tables hold h only (256/256/128 bf16 elems = 512/512/256B rows,
    the dma_gather 256B-granularity minimum). als = h@a_src is computed
    on-device from the gathered rows (DVE mult + reduce); ald comes from the
    block-local node rows via a transposed-one-hot matmul (PE). No second
    (dst-side) gather at all.
  - Tables are split in two halves (A: slice rows < 3200, B: rest), which
    both keeps gather indices < 32768 (int16) and lets each half's AllGather
    start mid node-phase (A issued after block 24, B after block 48),
    overlapping the collective with compute.
  - Gathers read the Shared AllGather outputs directly (measured as fast as
    Internal DRAM at these row sizes).
  - Softmax is denominator-style (exactly segment softmax, no segment-max
    needed at these logit scales).
  - Decoder: trans_bias gathers+reduction run before the TD-dependent
    z-gathers so they don't stall behind the final collectives.
"""
import numpy as np
import ml_dtypes

N = 50000
NP = 50176            # padded nodes: 8 * 6272
SLICE = NP // 8       # 6272 nodes per core
NB = SLICE // 128     # 49 blocks per core
HSLA = 3200           # A-half rows per slice (25 blocks)
HSLB = SLICE - HSLA   # B-half rows per slice (24 blocks)
NBA = HSLA // 128     # 25
NROWA = 8 * HSLA      # 25600 rows in table A
NROWB = 8 * HSLB      # 24576 rows in table B
E = 800000
EL = 100000
NTYPES = 311
EMB = 16
COMB = 48

# per-layer: (Din, HD, H, D); table width for layer l's edge phase = HD
LCFG = [
    (48, 256, 4, 64),
    (256, 256, 2, 128),
    (256, 128, 1, 128),
]
TDEC_W = 128          # decoder table row elems
TB_W = 384            # padded trans_bias row
DEC_TILE = 512
GMAX = 6              # max 128-chunks per dma_gather (descriptor-ring bound)
SKIP_COLL = False     # debug: skip collectives (timing only)
SKIP_GATHER = False   # debug: skip edge-phase gathers (timing only)

bf16 = ml_dtypes.bfloat16


def _wrap_idx(vals):
    """int16 index array for dma_gather: [128, n/16], wrapped over 16
    partitions and replicated across the 8 gpsimd cores."""
    n = len(vals)
    assert n % 16 == 0
    a = np.zeros((128, n // 16), np.int16)
    v = np.asarray(vals, np.int64)
    assert v.min() >= 0 and v.max() < 32768, (v.min(), v.max())
    w = v.reshape(n // 16, 16).T.astype(np.int16)  # [16, n/16]
    for g in range(8):
        a[16 * g:16 * g + 16, :] = w
    return a


def _slotmajor(vals, dtype):
    """[128, n/128] array with element (p, c) = vals[c*128+p]."""
    n = len(vals)
    assert n % 128 == 0
    return np.asarray(vals, np.float64).reshape(n // 128, 128).T.astype(dtype)


def prep(x, edge_index, edge_label_index, emb, W1, a_src1, a_dst1, b1,
         W2, a_src2, a_dst2, b2, W3, a_src3, a_dst3, b3,
         Wl1, bl1, Wl2, bl2, trans_bias):
    """Host-side (integer/index + weight-layout) preprocessing."""
    types = x[:, 0].astype(np.int64)

    # --- weight folds: RHS_l = [W_l | W_l@a_dst per head] ---
    def fold(W, a_d, H, D):
        cols_d = np.stack([W[:, h * D:(h + 1) * D] @ a_d[h] for h in range(H)], 1)
        return np.concatenate([W, cols_d], 1).astype(np.float32)
    RHS = [fold(W1, a_dst1, 4, 64),
           fold(W2, a_dst2, 2, 128),
           fold(W3, a_dst3, 1, 128)]
    ASRC = [np.tile(a.reshape(1, -1), (128, 1)).astype(bf16)
            for a in (a_src1, a_src2, a_src3)]

    emb_pad = np.zeros((NTYPES, 64), np.float32)
    emb_pad[:, :EMB] = emb
    TBpad = np.zeros((NTYPES, TB_W), bf16)
    TBpad[:, :NTYPES] = trans_bias.astype(bf16)

    # --- edges: add self loops, sort by dst, bucket per core / block ---
    loops = np.arange(N, dtype=np.int64)
    src = np.concatenate([edge_index[0].astype(np.int64), loops])
    dst = np.concatenate([edge_index[1].astype(np.int64), loops])
    order = np.argsort(dst, kind="stable")
    src, dst = src[order], dst[order]

    sown = src // SLICE
    srem = src % SLICE
    inA = srem < HSLA
    rowA = sown * HSLA + srem            # valid where inA
    rowB = sown * HSLB + (srem - HSLA)   # valid where ~inA

    blk = dst // 128          # global dst block id (0..391)
    per = [[None] * NB for _ in range(8)]
    for c in range(8):
        for b in range(NB):
            m = blk == c * NB + b
            per[c][b] = (rowA[m & inA], dst[m & inA],
                         rowB[m & ~inA], dst[m & ~inA])
    CA = [max(1, max((len(per[c][b][0]) + 127) // 128 for c in range(8)))
          for b in range(NB)]
    CB = [max((len(per[c][b][2]) + 127) // 128 for c in range(8))
          for b in range(NB)]

    idxA, idxB, doff = [], [], []
    for c in range(8):
        la, lb, lo = [], [], []
        for b in range(NB):
            base = (c * NB + b) * 128
            for half, cnt in ((0, CA[b]), (1, CB[b])):
                rows = per[c][b][0 + 2 * half]
                dsts = per[c][b][1 + 2 * half]
                ns = cnt * 128
                sp = np.zeros(ns, np.int64)
                sp[:len(rows)] = rows
                (la if half == 0 else lb).append(sp)
                off = np.full(ns, 255, np.int64)
                off[:len(dsts)] = dsts - base
                lo.append(off)
        idxA.append(_wrap_idx(np.concatenate(la)))
        idxB.append(_wrap_idx(np.concatenate(lb)))
        doff.append(_slotmajor(np.concatenate(lo), bf16))

    # --- label edges: 4 groups by (ls-half, ld-half), padded per group ---
    ls = edge_label_index[0].astype(np.int64)
    ld_ = edge_label_index[1].astype(np.int64)
    elpc = (EL + 7) // 8
    groups_sz = np.zeros((8, 4), np.int64)
    per_dec = [[None] * 4 for _ in range(8)]
    lsA = (ls % SLICE) < HSLA
    ldA = (ld_ % SLICE) < HSLA
    lsrow = np.where(lsA, (ls // SLICE) * HSLA + ls % SLICE,
                     (ls // SLICE) * HSLB + ls % SLICE - HSLA)
    ldrow = np.where(ldA, (ld_ // SLICE) * HSLA + ld_ % SLICE,
                     (ld_ // SLICE) * HSLB + ld_ % SLICE - HSLA)
    for c in range(8):
        lo_, hi_ = c * elpc, min((c + 1) * elpc, EL)
        eidx = np.arange(lo_, hi_)
        g = (~lsA[eidx]).astype(np.int64) * 2 + (~ldA[eidx])
        for gi in range(4):
            per_dec[c][gi] = eidx[g == gi]
            groups_sz[c, gi] = len(per_dec[c][gi])
    GSZ = [int(-(-groups_sz[:, gi].max() // DEC_TILE) * DEC_TILE)
           for gi in range(4)]
    SL = sum(GSZ)
    lsw, ldw, tlsw, tldw, slotmap = [], [], [], [], []
    for c in range(8):
        a_ls = np.zeros(SL, np.int64)
        a_ld = np.zeros(SL, np.int64)
        a_tls = np.zeros(SL, np.int64)
        a_tld = np.zeros(SL, np.int64)
        smap = np.full(SL, -1, np.int64)
        pos = 0
        for gi in range(4):
            e = per_dec[c][gi]
            n = len(e)
            a_ls[pos:pos + n] = lsrow[e]
            a_ld[pos:pos + n] = ldrow[e]
            a_tls[pos:pos + n] = types[np.minimum(ls[e], N - 1)]
            a_tld[pos:pos + n] = types[np.minimum(ld_[e], N - 1)]
            smap[pos:pos + n] = e
            pos += GSZ[gi]
        lsw.append(_wrap_idx(a_ls))
        ldw.append(_wrap_idx(a_ld))
        tlsw.append(_wrap_idx(a_tls))
        tldw.append(_slotmajor(a_tld, np.float32))
        slotmap.append(smap)

    iota128 = np.tile(np.arange(128, dtype=np.float64)[None, :],
                      (128, 1)).astype(bf16)
    iota384 = np.tile(np.arange(TB_W, dtype=np.float32)[None, :], (128, 1))
    ident = np.eye(128, dtype=np.float32)
    identb = np.eye(128, dtype=bf16)

    in_maps = []
    offs = {}

    def _pack(parts):
        flat = [np.ascontiguousarray(a).reshape(-1) for _, a in parts]
        off = 0
        for (nm, _), f in zip(parts, flat):
            if nm not in offs:
                offs[nm] = (off, len(f))
            off += len(f)
        return np.concatenate(flat)

    for c in range(8):
        xr = np.zeros((SLICE, 33), np.float32)
        n0 = c * SLICE
        n1 = min((c + 1) * SLICE, N)
        if n1 > n0:
            xr[:n1 - n0] = x[n0:n1]
        ti = np.zeros(SLICE, np.int64)
        if n1 > n0:
            ti[:n1 - n0] = types[n0:n1]
        packf = _pack([("x_rows", xr), ("emb_pad", emb_pad),
                       ("RHS1", RHS[0]), ("RHS2", RHS[1]), ("RHS3", RHS[2]),
                       ("tld", tldw[c]), ("iota384", iota384),
                       ("ident", ident),
                       ("bl1", bl1.reshape(64, 1).astype(np.float32))])
        packi = _pack([("embt_idx", _wrap_idx(ti)), ("idxA", idxA[c]),
                       ("idxB", idxB[c]), ("ls_idx", lsw[c]),
                       ("ld_idx", ldw[c]), ("tls_idx", tlsw[c])])
        packb = _pack([("asrc1", ASRC[0]), ("asrc2", ASRC[1]),
                       ("asrc3", ASRC[2]), ("doff", doff[c]),
                       ("iota128", iota128), ("identb", identb),
                       ("TBpad", TBpad), ("Wl1a", Wl1[:128].astype(bf16)),
                       ("Wl1b", Wl1[128:].astype(bf16)),
                       ("Wl2", Wl2.astype(bf16))])
        in_maps.append(dict(packf=packf.astype(np.float32),
                            packi=packi.astype(np.int16),
                            packb=packb.astype(bf16)))
    cfg = dict(CA=CA, CB=CB, GSZ=GSZ, SL=SL, offs=offs,
               sizes=dict(packf=len(in_maps[0]["packf"]),
                          packi=len(in_maps[0]["packi"]),
                          packb=len(in_maps[0]["packb"])),
               SA=sum(CA) * 128, SB=sum(CB) * 128,
               ST=(sum(CA) + sum(CB)) * 128,
               bl2=float(np.asarray(bl2).reshape(-1)[0]),
               b=[np.asarray(b1), np.asarray(b2), np.asarray(b3)],
               slotmap=slotmap)
    return in_maps, cfg


# ---------------------------------------------------------------- golden ---
def _unpack(im, cfg, name, shape, dtype):
    off, n = cfg["offs"][name]
    blob = {np.float32: "packf", np.int16: "packi"}.get(dtype, "packb")
    if dtype == bf16:
        blob = "packb"
    return im[blob][off:off + n].reshape(shape)


def golden(in_maps, cfg):
    """numpy mirror of the device algorithm (fp32; layout-accurate)."""
    CA, CB = cfg["CA"], cfg["CB"]
    SLICE_SH = (SLICE, 33)
    ims = []
    for im in in_maps:
        ims.append(dict(
            x_rows=_unpack(im, cfg, "x_rows", (SLICE, 33), np.float32),
            emb_pad=_unpack(im, cfg, "emb_pad", (NTYPES, 64), np.float32),
            RHS1=_unpack(im, cfg, "RHS1", (48, 260), np.float32),
            RHS2=_unpack(im, cfg, "RHS2", (256, 258), np.float32),
            RHS3=_unpack(im, cfg, "RHS3", (256, 129), np.float32),
            tld=_unpack(im, cfg, "tld", (128, cfg["SL"] // 128), np.float32),
            bl1=_unpack(im, cfg, "bl1", (64, 1), np.float32),
            embt_idx=_unpack(im, cfg, "embt_idx", (128, SLICE // 16), np.int16),
            idxA=_unpack(im, cfg, "idxA", (128, cfg["SA"] // 16), np.int16),
            idxB=_unpack(im, cfg, "idxB", (128, cfg["SB"] // 16), np.int16),
            ls_idx=_unpack(im, cfg, "ls_idx", (128, cfg["SL"] // 16), np.int16),
            ld_idx=_unpack(im, cfg, "ld_idx", (128, cfg["SL"] // 16), np.int16),
            tls_idx=_unpack(im, cfg, "tls_idx", (128, cfg["SL"] // 16), np.int16),
            asrc1=_unpack(im, cfg, "asrc1", (128, 256), bf16),
            asrc2=_unpack(im, cfg, "asrc2", (128, 256), bf16),
            asrc3=_unpack(im, cfg, "asrc3", (128, 128), bf16),
            doff=_unpack(im, cfg, "doff", (128, cfg["ST"] // 128), bf16),
            TBpad=_unpack(im, cfg, "TBpad", (NTYPES, TB_W), bf16),
            Wl1a=_unpack(im, cfg, "Wl1a", (128, 64), bf16),
            Wl1b=_unpack(im, cfg, "Wl1b", (128, 64), bf16),
            Wl2=_unpack(im, cfg, "Wl2", (64, 1), bf16),
        ))
    in_maps = ims
    out_all = None
    # tables per layer: TA [NROWA, W], TB [NROWB, W]
    PREV = None
    for li, (Din, HD, H, D) in enumerate(LCFG):
        TA = np.zeros((NROWA, HD), np.float32)
        TB = np.zeros((NROWB, HD), np.float32)
        ALD = []   # per core [SLICE, H]
        for c in range(8):
            im = in_maps[c]
            if li == 0:
                embg = im["emb_pad"][_unwrap(im["embt_idx"], SLICE)][:, :EMB]
                x0 = np.concatenate([embg, im["x_rows"][:, 1:33]], 1)
                pn = x0 @ im["RHS1"]
            else:
                pn = PREV[c] @ im[f"RHS{li + 1}"]
            h = pn[:, :HD].astype(bf16).astype(np.float32)
            ald = pn[:, HD:HD + H].astype(bf16).astype(np.float32)
            TA[c * HSLA:(c + 1) * HSLA] = h[:HSLA]
            TB[c * HSLB:(c + 1) * HSLB] = h[HSLA:]
            ALD.append(ald)
        PREV = []
        for c in range(8):
            im = in_maps[c]
            ia = _unwrap(im["idxA"], cfg["SA"])
            ib = _unwrap(im["idxB"], cfg["SB"])
            dof = im["doff"].astype(np.float64).T.reshape(-1)  # slot order
            asrc = im[f"asrc{li + 1}"][0].astype(np.float32)   # [HD]
            xl = np.zeros((SLICE, HD), np.float32)
            pa = pb = pt = 0
            for b in range(NB):
                sA, sB = CA[b] * 128, CB[b] * 128
                nsl = sA + sB
                G = np.concatenate([TA[ia[pa:pa + sA]], TB[ib[pb:pb + sB]]])
                pa += sA; pb += sB
                off = dof[pt:pt + nsl].astype(np.int64)
                pt += nsl
                S = (off[:, None] == np.arange(128)[None, :])
                amul = (G * asrc[None, :]).astype(bf16).astype(np.float32)
                als = amul.reshape(nsl, H, D).sum(2)              # [nsl,H]
                aldb = ALD[c][b * 128:(b + 1) * 128]              # [128,H]
                ald_slot = S.astype(np.float32) @ aldb            # [nsl,H]
                u = als + ald_slot
                lg = np.where(u > 0, u, 0.2 * u)
                e = np.exp(lg).astype(bf16).astype(np.float32)
                msg = (G.reshape(nsl, H, D) * e[:, :, None]
                       ).reshape(nsl, HD).astype(bf16).astype(np.float32)
                num = S.T.astype(np.float32) @ msg
                den = S.T.astype(np.float32) @ e
                r = 1.0 / (den + 1e-16)
                xb = (num.reshape(128, H, D) * r[:, :, None]).reshape(128, HD)
                bvec = cfg["b"][li]
                if np.any(bvec != 0):
                    xb = xb + bvec
                if li < 2:
                    xb = np.maximum(xb, 0) + np.exp(np.minimum(xb, 0)) - 1
                xl[b * 128:(b + 1) * 128] = xb
            PREV.append(xl)
    # decode tables
    TDA = np.zeros((NROWA, TDEC_W), np.float32)
    TDB = np.zeros((NROWB, TDEC_W), np.float32)
    for c in range(8):
        z = PREV[c].astype(bf16).astype(np.float32)
        TDA[c * HSLA:(c + 1) * HSLA] = z[:HSLA]
        TDB[c * HSLB:(c + 1) * HSLB] = z[HSLA:]
    scores = []
    for c in range(8):
        im = in_maps[c]
        lsv = _unwrap(im["ls_idx"], cfg["SL"])
        ldv = _unwrap(im["ld_idx"], cfg["SL"])
        tlsv = _unwrap(im["tls_idx"], cfg["SL"])
        tldv = im["tld"].T.reshape(-1)
        zl = np.zeros((cfg["SL"], TDEC_W), np.float32)
        zr = np.zeros((cfg["SL"], TDEC_W), np.float32)
        pos = 0
        for gi in range(4):
            sl_ = slice(pos, pos + cfg["GSZ"][gi])
            zl[sl_] = (TDB if gi >= 2 else TDA)[lsv[sl_]]
            zr[sl_] = (TDB if gi % 2 else TDA)[ldv[sl_]]
            pos += cfg["GSZ"][gi]
        W1a = im["Wl1a"].astype(np.float32)
        W1b = im["Wl1b"].astype(np.float32)
        h = np.maximum(zl @ W1a + zr @ W1b + im["bl1"].T, 0
                       ).astype(bf16).astype(np.float32)
        base = h @ im["Wl2"].astype(np.float32) + cfg["bl2"]
        TBg = im["TBpad"].astype(np.float32)[tlsv]
        oh = (tldv[:, None] == np.arange(TB_W)[None, :])
        bias = (TBg * oh).sum(1)
        scores.append(base[:, 0] + bias)
    out = np.zeros((EL, 1), np.float32)
    for c in range(8):
        m = cfg["slotmap"][c] >= 0
        out[cfg["slotmap"][c][m], 0] = scores[c][m]
    return out


def _unwrap(w, n):
    return w[:16, :].T.reshape(-1)[:n].astype(np.int64)


# ----------------------------------------------------------------- device ---
def build(cfg):
    import concourse.bacc as bacc
    import concourse.mybir as mybir
    from concourse.tile import TileContext
    dt = mybir.dt
    F = mybir.ActivationFunctionType
    A = mybir.AluOpType
    CA, CB, SL = cfg["CA"], cfg["CB"], cfg["SL"]
    SA, SB, ST = cfg["SA"], cfg["SB"], cfg["ST"]
    TBLW = [256, 256, 128]

    nc = bacc.Bacc(num_devices=8, dynamic_dma_scratch_size=32768)

    def gat(out_ap, in_ap, idx_tile, col0, nchunk, elem, **kw):
        for s0 in range(0, nchunk, GMAX):
            s1 = min(s0 + GMAX, nchunk)
            nc.gpsimd.dma_gather(
                out_ap[:, s0:s1, :], in_ap,
                idx_tile[:, col0 + s0 * 8: col0 + s1 * 8],
                (s1 - s0) * 128, (s1 - s0) * 128, elem, **kw)

    packs = {
        "packf": nc.dram_tensor("packf", [cfg["sizes"]["packf"]], dt.float32,
                                kind="ExternalInput"),
        "packi": nc.dram_tensor("packi", [cfg["sizes"]["packi"]], dt.int16,
                                kind="ExternalInput"),
        "packb": nc.dram_tensor("packb", [cfg["sizes"]["packb"]], dt.bfloat16,
                                kind="ExternalInput"),
    }
    _blob_of = dict(
        x_rows="packf", emb_pad="packf", RHS1="packf", RHS2="packf",
        RHS3="packf", tld="packf", iota384="packf", ident="packf",
        bl1="packf",
        embt_idx="packi", idxA="packi", idxB="packi", ls_idx="packi",
        ld_idx="packi", tls_idx="packi",
        asrc1="packb", asrc2="packb", asrc3="packb", doff="packb",
        iota128="packb", identb="packb", TBpad="packb", Wl1a="packb",
        Wl1b="packb", Wl2="packb")

    class _Inp:
        def __getitem__(self, name):
            off, n = cfg["offs"][name]
            return packs[_blob_of[name]][off:off + n]
    inp = _Inp()
    score_out = nc.dram_tensor("score", [SL, 1], dt.float32, kind="ExternalOutput")

    slA = [nc.dram_tensor(f"slA{l}", [HSLA, TBLW[l]], dt.bfloat16,
                          kind="Internal") for l in range(3)]
    slB = [nc.dram_tensor(f"slB{l}", [HSLB, TBLW[l]], dt.bfloat16,
                          kind="Internal") for l in range(3)]
    slDA = nc.dram_tensor("slDA", [HSLA, TDEC_W], dt.bfloat16, kind="Internal")
    slDB = nc.dram_tensor("slDB", [HSLB, TDEC_W], dt.bfloat16, kind="Internal")
    T_A = [nc.dram_tensor(f"TA{l}", [NROWA, TBLW[l]], dt.bfloat16,
                          kind="Internal", addr_space="Shared") for l in range(3)]
    T_B = [nc.dram_tensor(f"TB{l}", [NROWB, TBLW[l]], dt.bfloat16,
                          kind="Internal", addr_space="Shared") for l in range(3)]
    TD_A = nc.dram_tensor("TDA", [NROWA, TDEC_W], dt.bfloat16,
                          kind="Internal", addr_space="Shared")
    TD_B = nc.dram_tensor("TDB", [NROWB, TDEC_W], dt.bfloat16,
                          kind="Internal", addr_space="Shared")

    def allgather(src, dst):
        if SKIP_COLL:
            return
        nc.gpsimd.collective_compute(
            "AllGather", mybir.AluOpType.bypass,
            ins=[src[:]], outs=[dst[:]],
            replica_groups=[list(range(8))])

    with TileContext(nc, num_cores=8) as tc:
        with tc.tile_pool(name="const", bufs=1) as cpool:
            def load(name, shape, d):
                t = cpool.tile(shape, d, tag=name, name=name)
                v = inp[name]
                if len(shape) == 2:
                    v = v.rearrange("(p w) -> p w", w=shape[1])
                nc.sync.dma_start(t[:], v)
                return t
            idxA = load("idxA", [128, SA // 16], dt.int16)
            idxB = load("idxB", [128, SB // 16], dt.int16)
            doff = load("doff", [128, ST // 128], dt.bfloat16)
            iota = load("iota128", [128, 128], dt.bfloat16)
            ident = load("ident", [128, 128], dt.float32)
            identb = load("identb", [128, 128], dt.bfloat16)
            RHSs = [load("RHS1", [48, 260], dt.float32)]
            for l, w in ((2, 258), (3, 129)):
                t = cpool.tile([128, 2, w], dt.float32, tag=f"RHS{l}",
                               name=f"RHS{l}t")
                nc.sync.dma_start(
                    t[:], inp[f"RHS{l}"].rearrange("(k p w) -> p k w",
                                                   p=128, w=w))
                RHSs.append(t)
            asrc = [load("asrc1", [128, 256], dt.bfloat16),
                    load("asrc2", [128, 256], dt.bfloat16),
                    load("asrc3", [128, 128], dt.bfloat16)]
            aldres = [cpool.tile([128, NB, h], dt.bfloat16, tag=f"aldres{l}",
                                 name=f"aldres{l}")
                      for l, h in ((0, 4), (1, 2), (2, 1))]

            wpool = tc.alloc_tile_pool(name="work", bufs=2)
            ppool = tc.alloc_tile_pool(name="psum", bufs=2, space="PSUM")
            ppool1 = tc.alloc_tile_pool(name="psum1", bufs=1, space="PSUM")

            def row_out(b, pn_ap, li_next, HDn, Hn):
                """copy node-phase PSUM to bf16 row, DMA to slA/slB, stash ald."""
                if li_next < 3:
                    row = wpool.tile([128, HDn], dt.bfloat16, tag="row")
                    nc.scalar.activation(row[:], pn_ap[:, 0:HDn], F.Copy)
                    nc.vector.tensor_copy(aldres[li_next][:, b, :],
                                          pn_ap[:, HDn:HDn + Hn])
                    dstA, dstB = slA[li_next], slB[li_next]
                else:
                    row = wpool.tile([128, TDEC_W], dt.bfloat16, tag="row")
                    nc.scalar.activation(row[:], pn_ap[:], F.Copy)
                    dstA, dstB = slDA, slDB
                if b < NBA:
                    nc.sync.dma_start(dstA[b * 128:(b + 1) * 128, :], row[:])
                else:
                    b2 = b - NBA
                    nc.sync.dma_start(dstB[b2 * 128:(b2 + 1) * 128, :], row[:])

            # ---- prologue: x0 = [emb | x], layer-1 table rows ----
            embg = cpool.tile([128, NB, 64], dt.float32, tag="embg")
            embt = load("embt_idx", [128, SLICE // 16], dt.int16)
            gat(embg, inp["emb_pad"].rearrange("(a b) -> a b", b=64),
                embt[:], 0, NB, 64)
            for b in range(NB):
                xb = wpool.tile([128, 33], dt.float32, tag="xb")
                nc.sync.dma_start(
                    xb[:], inp["x_rows"][b * 128 * 33:(b + 1) * 128 * 33]
                    .rearrange("(p w) -> p w", w=33))
                x0 = wpool.tile([128, 48], dt.float32, tag="x0")
                nc.vector.tensor_copy(x0[:, 0:EMB], embg[:, b, 0:EMB])
                nc.vector.tensor_copy(x0[:, EMB:48], xb[:, 1:33])
                pt = ppool1.tile([128, 128], dt.float32, tag="pt")
                nc.tensor.transpose(pt[0:48, :], x0[:], ident[:])
                x0T = wpool.tile([48, 128], dt.float32, tag="x0T")
                nc.scalar.activation(x0T[:], pt[0:48, :], F.Copy)
                pn = ppool1.tile([128, 260], dt.float32, tag="pn")
                nc.tensor.matmul(pn[:, 0:260], x0T[:], RHSs[0][:],
                                 start=True, stop=True)
                row_out(b, pn, 0, 256, 4)
                if b == NBA - 1:
                    allgather(slA[0], T_A[0])
            allgather(slB[0], T_B[0])

            # ---- three GAT layers ----
            for li, (Din, HD, H, D) in enumerate(LCFG):
                RW = HD + H
                pa = pb = pt_ = 0
                for b in range(NB):
                    cA, cB = CA[b], CB[b]
                    C = cA + cB
                    G = wpool.tile([128, C, HD], dt.bfloat16, tag="G")
                    if not SKIP_GATHER:
                        gat(G, T_A[li][:], idxA[:], pa // 16, cA, HD)
                        if cB:
                            gat(G[:, cA:C, :], T_B[li][:], idxB[:],
                                pb // 16, cB, HD)
                    # als = per-head <h, a_src>: mult into RT scratch, reduce
                    RT = wpool.tile([128, C, RW], dt.bfloat16, tag="RT")
                    nc.vector.tensor_tensor(
                        RT[:, :, 0:HD],
                        G[:],
                        asrc[li][:].unsqueeze(1).broadcast_to([128, C, HD]),
                        A.mult)
                    als = wpool.tile([128, C, H], dt.float32, tag="als")
                    nc.vector.tensor_reduce(
                        als[:],
                        RT[:, :, 0:HD].rearrange("p c (h d) -> p c h d", h=H),
                        mybir.AxisListType.X, A.add)
                    # one-hot S (slot -> dst offset)
                    S = wpool.tile([128, C, 128], dt.bfloat16, tag="S")
                    nc.vector.tensor_tensor(
                        S[:],
                        doff[:, pt_ // 128: pt_ // 128 + C].unsqueeze(-1)
                            .broadcast_to([128, C, 128]),
                        iota[:].unsqueeze(1).broadcast_to([128, C, 128]),
                        A.is_equal)
                    # ald per slot: transpose S chunks, matmul with ald block
                    st_all = wpool.tile([128, C, 128], dt.bfloat16, tag="st")
                    ups = ppool1.tile([128, C, H], dt.float32, tag="ups")
                    for ch in range(C):
                        stp = ppool.tile([128, 128], dt.bfloat16, tag="stp")
                        nc.tensor.transpose(stp[:], S[:, ch, :], identb[:])
                        nc.scalar.activation(st_all[:, ch, :], stp[:], F.Copy)
                        nc.tensor.matmul(ups[:, ch, :], st_all[:, ch, :],
                                         aldres[li][:, b, :],
                                         start=True, stop=True)
                    u = wpool.tile([128, C, H], dt.float32, tag="u")
                    nc.vector.tensor_tensor(u[:], als[:], ups[:], A.add)
                    nc.scalar.activation(u[:], u[:], F.Lrelu, alpha=0.2)
                    nc.scalar.activation(RT[:, :, HD:HD + H], u[:], F.Exp)
                    # messages = h * e
                    nc.vector.tensor_tensor(
                        RT[:, :, 0:HD].rearrange("p c (h d) -> p c h d", h=H),
                        G[:].rearrange("p c (h d) -> p c h d", h=H),
                        RT[:, :, HD:HD + H].unsqueeze(-1)
                            .broadcast_to([128, C, H, D]),
                        A.mult)
                    pe = ppool.tile([128, RW], dt.float32, tag="pe")
                    for ch in range(C):
                        nc.tensor.matmul(pe[:, 0:RW], S[:, ch, :], RT[:, ch, :],
                                         start=(ch == 0), stop=(ch == C - 1))
                    pa += cA * 128
                    pb += cB * 128
                    pt_ += C * 128
                    # ---- finalize + node phase ----
                    den = wpool.tile([128, H], dt.float32, tag="den")
                    nc.vector.tensor_scalar_add(den[:], pe[:, HD:HD + H], 1e-16)
                    rec = wpool.tile([128, H], dt.float32, tag="rec")
                    nc.vector.reciprocal(rec[:], den[:])
                    xo = wpool.tile([128, HD], dt.float32, tag="xo")
                    nc.vector.tensor_tensor(
                        xo[:].rearrange("p (h d) -> p h d", h=H),
                        pe[:, 0:HD].rearrange("p (h d) -> p h d", h=H),
                        rec[:].unsqueeze(-1).broadcast_to([128, H, D]),
                        A.mult)
                    if li < 2:
                        m = wpool.tile([128, HD], dt.float32, tag="melu")
                        nc.vector.tensor_scalar_min(m[:], xo[:], 0.0)
                        nc.scalar.activation(m[:], m[:], F.Exp)
                        nc.vector.tensor_scalar_max(xo[:], xo[:], 0.0)
                        nc.vector.tensor_tensor(xo[:], xo[:], m[:], A.add)
                        nc.vector.tensor_scalar_add(xo[:], xo[:], -1.0)
                        NHD, NH = LCFG[li + 1][1], LCFG[li + 1][2]
                        NW = NHD + NH
                        xT = wpool.tile([128, 2, 128], dt.float32, tag="xT")
                        for kc in range(2):
                            ptp = ppool1.tile([128, 128], dt.float32, tag="pt")
                            nc.tensor.transpose(
                                ptp[:], xo[:, kc * 128:(kc + 1) * 128], ident[:])
                            nc.scalar.activation(xT[:, kc, :], ptp[:], F.Copy)
                        pn = ppool1.tile([128, 260], dt.float32, tag="pn")
                        for kc in range(2):
                            nc.tensor.matmul(pn[:, 0:NW], xT[:, kc, :],
                                             RHSs[li + 1][:, kc, :],
                                             start=(kc == 0), stop=(kc == 1))
                        row_out(b, pn, li + 1, NHD, NH)
                    else:
                        row_out(b, xo, 3, TDEC_W, 0)
                    if b == NBA - 1:
                        if li < 2:
                            allgather(slA[li + 1], T_A[li + 1])
                        else:
                            allgather(slDA, TD_A)
                if li < 2:
                    allgather(slB[li + 1], T_B[li + 1])
                else:
                    allgather(slDB, TD_B)

            ppool1.release()
            ppool.release()
            wpool.release()
            wpool = tc.alloc_tile_pool(name="dwork", bufs=2)
            ppool = tc.alloc_tile_pool(name="dpsum", bufs=2, space="PSUM")

            # ---- decoder ----
            iota384 = load("iota384", [128, TB_W], dt.float32)
            lsi = load("ls_idx", [128, SL // 16], dt.int16)
            ldi = load("ld_idx", [128, SL // 16], dt.int16)
            tlsi = load("tls_idx", [128, SL // 16], dt.int16)
            tld = load("tld", [128, SL // 128], dt.float32)
            W1a = load("Wl1a", [128, 64], dt.bfloat16)
            W1b = load("Wl1b", [128, 64], dt.bfloat16)
            W2d = load("Wl2", [64, 1], dt.bfloat16)
            bl1 = load("bl1", [64, 1], dt.float32)
            score_sb = cpool.tile([128, SL // 128], dt.float32, tag="score")
            biases = cpool.tile([128, SL // 128], dt.float32, tag="biases")
            # pass 1: trans_bias (independent of the TD collectives)
            for t0 in range(0, SL, DEC_TILE):
                TBg = wpool.tile([128, 4, TB_W], dt.bfloat16, tag="TBg")
                nc.gpsimd.dma_gather(
                    TBg[:], inp["TBpad"].rearrange("(a b) -> a b", b=TB_W),
                    tlsi[:, t0 // 16:(t0 + DEC_TILE) // 16],
                    DEC_TILE, DEC_TILE, TB_W)
                oh = wpool.tile([128, 4, TB_W], dt.bfloat16, tag="oh")
                nc.vector.tensor_tensor(
                    oh[:],
                    tld[:, t0 // 128: t0 // 128 + 4].unsqueeze(-1)
                        .broadcast_to([128, 4, TB_W]),
                    iota384[:].unsqueeze(1).broadcast_to([128, 4, TB_W]),
                    A.is_equal)
                tb2 = wpool.tile([128, 4, TB_W], dt.bfloat16, tag="tb2")
                nc.vector.tensor_tensor(tb2[:], TBg[:], oh[:], A.mult)
                nc.vector.tensor_reduce(
                    biases[:, t0 // 128: t0 // 128 + 4], tb2[:],
                    mybir.AxisListType.X, A.add)
            # pass 2: z gathers + MLP
            pos = 0
            for gi in range(4):
                Tsrc = TD_B if gi >= 2 else TD_A
                Tdst = TD_B if gi % 2 else TD_A
                for t0 in range(pos, pos + cfg["GSZ"][gi], DEC_TILE):
                    zl = wpool.tile([128, 1, DEC_TILE], dt.bfloat16, tag="zl")
                    nc.gpsimd.dma_gather(
                        zl[:], Tsrc[:], lsi[:, t0 // 16:(t0 + DEC_TILE) // 16],
                        DEC_TILE, DEC_TILE, TDEC_W, transpose=True)
                    zr = wpool.tile([128, 1, DEC_TILE], dt.bfloat16, tag="zr")
                    nc.gpsimd.dma_gather(
                        zr[:], Tdst[:], ldi[:, t0 // 16:(t0 + DEC_TILE) // 16],
                        DEC_TILE, DEC_TILE, TDEC_W, transpose=True)
                    ph = ppool.tile([64, DEC_TILE], dt.float32, tag="ph")
                    nc.tensor.matmul(ph[:], W1a[:], zl[:, 0, :],
                                     start=True, stop=False)
                    nc.tensor.matmul(ph[:], W1b[:], zr[:, 0, :],
                                     start=False, stop=True)
                    hd = wpool.tile([64, DEC_TILE], dt.bfloat16, tag="hd")
                    nc.scalar.activation(hd[:], ph[:], F.Relu, bias=bl1[:])
                    for sub in range(4):
                        pss = ppool.tile([128, 1], dt.float32, tag="pss")
                        nc.tensor.matmul(pss[:], hd[:, sub * 128:(sub + 1) * 128],
                                         W2d[:], start=True, stop=True)
                        col = t0 // 128 + sub
                        nc.vector.tensor_tensor(
                            score_sb[:, col:col + 1], pss[:],
                            biases[:, col:col + 1], A.add)
                pos += cfg["GSZ"][gi]
            if cfg["bl2"] != 0.0:
                nc.vector.tensor_scalar_add(score_sb[:], score_sb[:], cfg["bl2"])
            nc.sync.dma_start(
                score_out[:].rearrange("(c p) o -> p (c o)", p=128), score_sb[:])
            ppool.release()
            wpool.release()
    nc.finalize()
    return nc


def kernel(**inputs):
    inputs = {k: np.asarray(v) for k, v in inputs.items()}
    in_maps, cfg = prep(**inputs)
    nc = build(cfg)
    from concourse.bass_utils import run_bass_kernel_spmd
    res = run_bass_kernel_spmd(nc, in_maps, core_ids=list(range(8)))
    out = np.zeros((EL, 1), np.float32)
    for c in range(8):
        sc = res.results[c]["score"][:, 0]
        m = cfg["slotmap"][c] >= 0
        out[cfg["slotmap"][c][m], 0] = sc[m]
    return out
